# revision 8
# baseline (speedup 1.0000x reference)
"""Sequence-parallel attention kernel for 8 Trainium2 NeuronCores.

Problem: nn_Attention_v2 — QKV projections + softmax attention + out-proj.
  query [2048,256], key/value [16384,256], weights [256,256], H=8 heads, KD=VD=32.

Sharding: K/V sequence split 8 ways (2048 rows/core); query replicated.
Each core computes, for all 8 heads, the *unnormalized* attention numerator
Onum = exp(S) @ V and denominator l = exp(S) @ 1 over its local K/V chunk
(no max subtraction needed: logits ~ N(0,1), |S| < ~7, exp is safe in fp32).
A ReduceScatter sums (Onum, l) across cores and shards the result by query
columns; each core then divides, applies the output projection for its query
shard, and the host concatenates the 8 shards.

Fast path vs the original version:
- exp is split between the ACT engine (exact, LUT) and the DVE using a
  3-op bit-trick product approx (two phase-shifted Schraudolph factors,
  ~1.5% max rel err on ~5/16 of key chunks -> ~6e-3 output rel err,
  budget is 2e-2): i1 = int32(A*s+B); i2 = i1 + dI; P = bits(i1)*bits(i2).
- P and the value heads are bf16 so the AV matmuls can column-pack 4-wide
  (f32r matmuls require dst partition 0): heads at psum partitions 0/32,
  softmax denominators via ones-column matmuls at partitions 64/96 -- all
  four stream concurrently on separate column groups.
- S matmuls stay f32r (full speed, dst partition 0), 2-way row-packed.
"""
import sys

sys.path.insert(0, "/opt/trn_rl_repo")

import numpy as np

import concourse.bass as bass  # noqa: F401  (import order matters)
from concourse import bacc
import concourse.mybir as mybir
from concourse.bass_utils import run_bass_kernel_spmd
from concourse.tile import TileContext
from concourse.masks import make_identity

F32 = mybir.dt.float32
F32R = mybir.dt.float32r
BF16 = mybir.dt.bfloat16
I32 = mybir.dt.int32
EXP = mybir.ActivationFunctionType.Exp
MULT = mybir.AluOpType.mult
ADD = mybir.AluOpType.add

NC_CORES = 8
TQ, T, D = 2048, 16384, 256
H, KD, VD, DOUT = 8, 32, 32, 256
HD = H * KD  # 256
TLOC = T // NC_CORES          # 2048 local K/V rows
NKT = TLOC // 128             # 16 k-chunks
NJQ = TQ // 512               # 4 q-column chunks of 512
QG = 64                       # q columns per rank-group in the RS layout
SCALE = float(1.0 / np.sqrt(KD))

# DVE 3-op exp approximation constants (fitted; ~1.54% max rel err)
A_H = float(np.float32(2**22 * 1.4426950408889634))
B1 = float(np.float32(1066999681.734))
DI = -4193778


def dve_kk_set(b):
    """Which k-chunks of block b the DVE computes exp for (rest on ACT).
    Blocks 0-1 are ACT-only: the DVE is busy with prologue transposes."""
    if b < 2:
        return frozenset()
    return frozenset({1, 4, 7, 10, 13})


def build_nc():
    nc = bacc.Bacc("TRN2", target_bir_lowering=False)

    t_query = nc.dram_tensor("query", [TQ, D], F32, kind="ExternalInput")
    t_key = nc.dram_tensor("key", [TLOC, D], F32, kind="ExternalInput")
    t_value = nc.dram_tensor("value", [TLOC, D], F32, kind="ExternalInput")
    t_wq = nc.dram_tensor("wq", [D, HD], F32, kind="ExternalInput")
    t_wk = nc.dram_tensor("wk", [D, HD], F32, kind="ExternalInput")
    t_wv = nc.dram_tensor("wv", [D, HD], F32, kind="ExternalInput")
    t_wo = nc.dram_tensor("wo", [HD, DOUT], F32, kind="ExternalInput")
    t_bq = nc.dram_tensor("bq", [HD], F32, kind="ExternalInput")
    t_bk = nc.dram_tensor("bk", [HD], F32, kind="ExternalInput")
    t_bv = nc.dram_tensor("bv", [HD], F32, kind="ExternalInput")
    t_bo = nc.dram_tensor("bo", [DOUT], F32, kind="ExternalInput")
    t_out = nc.dram_tensor("out", [TQ // NC_CORES, DOUT], F32, kind="ExternalOutput")

    with TileContext(nc) as tc:
        with tc.tile_pool(name="const", bufs=1) as constp, \
             tc.tile_pool(name="persist", bufs=1) as persist, \
             tc.tile_pool(name="dram", bufs=1, space="DRAM") as dramp:

            ident = constp.tile([128, 128], F32)
            make_identity(nc, ident[:])
            ones_f = constp.tile([128, 1], F32)
            nc.gpsimd.memset(ones_f[:], 1.0)
            ones1 = constp.tile([128, 1], BF16)
            nc.vector.tensor_copy(ones1[:], ones_f[:])

            # projected, transposed activations (feature rows on partitions)
            qhT = [persist.tile([128, TQ], F32R, tag=f"qhT{m}", name=f"qhT{m}") for m in range(2)]
            khT = [persist.tile([128, TLOC], F32R, tag=f"khT{m}", name=f"khT{m}") for m in range(2)]
            vh = [persist.tile([128, 256], BF16, tag=f"vh{t}", name=f"vh{t}") for t in range(NKT)]

            with tc.tile_pool(name="tin", bufs=6) as tin, \
                 tc.tile_pool(name="tT", bufs=1) as tTp, \
                 tc.tile_pool(name="pS", bufs=3, space="PSUM") as pS, \
                 tc.tile_pool(name="pO", bufs=2, space="PSUM") as pO, \
                 tc.tile_pool(name="pbuf", bufs=8) as pbuf, \
                 tc.tile_pool(name="ipool", bufs=2) as ipool, \
                 tc.tile_pool(name="wstage", bufs=2) as wstage, \
                 tc.tile_pool(name="stage", bufs=2) as stage:

                def ps_tile():
                    # shared 3-deep PSUM ring: prologue transposes/projections
                    # and the attention S tiles all draw [128,1024] slots
                    return pS.tile([128, 1024], F32, tag="S", name="ps")

                # ---- weights + biases to SBUF (rounded to f32r; wq,bq pre-scaled).
                # One DMA per weight: [256,256] -> [128, 512] with D-chunk a in
                # cols 256a..; lhsT slice for (dc, m) = [:, 256*dc+128*m :+128].
                wcomb = {}

                def load_w(tdram, key, scale_mul, wdt):
                    raw = wstage.tile([128, 512], F32, tag="wraw", name="wraw")
                    nc.sync.dma_start(
                        out=raw[:].rearrange("p (a d) -> p a d", a=2),
                        in_=tdram[:].rearrange("(a p) d -> p a d", a=2))
                    wt = persist.tile([128, 512], wdt, tag=f"w_{key}", name=f"w_{key}")
                    if scale_mul is not None:
                        nc.vector.tensor_scalar_mul(wt[:], raw[:], scale_mul)
                    else:
                        nc.vector.tensor_copy(wt[:], raw[:])
                    wcomb[key] = wt
                    return [wt[:, 256 * dc:256 * (dc + 1)] for dc in range(2)]

                def load_b(tdram, dst, scale_mul, key):
                    braw = wstage.tile([128, 2], F32, tag="braw", name="braw")
                    nc.sync.dma_start(out=braw[:],
                                      in_=tdram[:].rearrange("(a p) -> p a", a=2))
                    bt = persist.tile([128, 2], F32, tag=f"b_{key}", name=f"b_{key}")
                    if scale_mul is not None:
                        nc.vector.tensor_scalar_mul(bt[:], braw[:], scale_mul)
                    else:
                        nc.vector.tensor_copy(bt[:], braw[:])
                    for m in range(2):
                        dst[m] = bt[:, m:m + 1]

                # transposed raw inputs, both D-chunks side by side: [128, 2*ntok]
                qT = tTp.tile([128, 2 * TQ], F32R, tag="qT", name="qT")
                kT = tTp.tile([128, 2 * TLOC], F32R, tag="kT", name="kT")
                vT = tTp.tile([128, 2 * TLOC], F32R, tag="vT", name="vT")

                def load_tile(tdram, dst, ncols, i, tag):
                    """DMA 128 rows of [tok,256], transpose both 128-col halves
                    into one psum tile, copy once into dst[:, dc*ncols + span]."""
                    raw = tin.tile([128, 256], F32, tag=f"in_{tag}", name=f"in_{tag}")
                    nc.sync.dma_start(out=raw[:], in_=tdram[i * 128:(i + 1) * 128, :])
                    pt = ps_tile()
                    for m in range(2):
                        nc.tensor.transpose(pt[:, m * 128:(m + 1) * 128],
                                            raw[:, m * 128:(m + 1) * 128], ident[:])
                    dview = dst[:].rearrange("p (a t) -> p a t", a=2)
                    nc.vector.tensor_copy(
                        dview[:, :, i * 128:(i + 1) * 128],
                        pt[:, 0:256].rearrange("p (a t) -> p a t", a=2))

                def kchunk(j):
                    for i in range(4 * j, 4 * j + 4):
                        load_tile(t_key, kT, TLOC, i, "kT")
                    for m in range(2):
                        pp = ps_tile()
                        for dc in range(2):
                            nc.tensor.matmul(pp[:, 0:512],
                                             wk_r[dc][:, m * 128:(m + 1) * 128],
                                             kT[:, dc * TLOC + j * 512:dc * TLOC + (j + 1) * 512],
                                             start=(dc == 0), stop=(dc == 1))
                        nc.vector.tensor_scalar_add(khT[m][:, j * 512:(j + 1) * 512],
                                                    pp[:, 0:512], bk_c[m])

                def qchunk(j):
                    for i in range(4 * j, 4 * j + 4):
                        load_tile(t_query, qT, TQ, i, "qT")
                    for m in range(2):
                        pp = ps_tile()
                        for dc in range(2):
                            nc.tensor.matmul(pp[:, 0:512],
                                             wq_r[dc][:, m * 128:(m + 1) * 128],
                                             qT[:, dc * TQ + j * 512:dc * TQ + (j + 1) * 512],
                                             start=(dc == 0), stop=(dc == 1))
                        nc.vector.tensor_scalar_add(qhT[m][:, j * 512:(j + 1) * 512],
                                                    pp[:, 0:512], bq_c[m])

                def vchunk(j):
                    for t in range(4 * j, 4 * j + 4):
                        load_tile(t_value, vT, TLOC, t, "vT")
                        pp = ps_tile()
                        for dc in range(2):
                            nc.tensor.matmul(pp[:, 0:256],
                                             vT[:, dc * TLOC + t * 128:dc * TLOC + (t + 1) * 128],
                                             wv_r[dc][:], start=(dc == 0), stop=(dc == 1))
                        nc.vector.tensor_add(vh[t][:], pp[:, 0:256], bv_rep[:])

                # streaming order: weights just before their first consumer so
                # the k0/q0 input DMAs aren't stuck behind all weight DMAs;
                # first block consumes k/v chunks at ~1 per 3.5us from t~5us;
                # q chunks for jq>=1 are only needed after 4 blocks (~60us)
                wk_r = load_w(t_wk, "wk", None, F32R)
                wq_r = load_w(t_wq, "wq", SCALE, F32R)
                bq_c, bk_c, bo_c = [None, None], [None, None], [None, None]
                load_b(t_bk, bk_c, None, "bk")
                load_b(t_bq, bq_c, SCALE, "bq")
                kchunk(0)
                qchunk(0)
                wv_r = load_w(t_wv, "wv", None, F32R)
                # bv replicated across partitions for the vh epilogue
                bv_row = persist.tile([1, 256], F32)
                nc.sync.dma_start(out=bv_row[:], in_=t_bv[:].rearrange("(a d) -> a d", a=1))
                bv_rep = persist.tile([128, 256], F32)
                nc.gpsimd.partition_broadcast(bv_rep[:], bv_row[0:1, :])
                vchunk(0)
                wo_r = load_w(t_wo, "wo", None, BF16)
                load_b(t_bo, bo_c, None, "bo")
                for j in range(1, 4):
                    kchunk(j)
                    vchunk(j)
                for j in range(1, 4):
                    qchunk(j)

                # ---- main attention loop + per-(jq, head-pair) ReduceScatter ----
                z_in = [[dramp.tile([NC_CORES, 66, QG], F32, tag=f"zin{j}_{p}", name=f"zin{j}_{p}")
                         for p in range(4)] for j in range(NJQ)]
                z_out = [[dramp.tile([66, QG], F32, tag=f"zout{j}_{p}", name=f"zout{j}_{p}")
                          for p in range(4)] for j in range(NJQ)]

                for jq in range(NJQ):
                    for pi in range(4):
                        b = 4 * jq + pi
                        dve_set = dve_kk_set(b)
                        h0 = 2 * pi
                        ti = h0 // 4
                        po0 = 32 * (h0 % 4)
                        # psO: head h0 rows at partitions 0-31, h1 at 32-63,
                        # softmax denominators at partitions 64 and 96.
                        psO = pO.tile([128, 512], F32, tag="O", name="psO")
                        for kk in range(NKT):
                            first, last = kk == 0, kk == NKT - 1
                            S = ps_tile()
                            for j in range(2):
                                po = po0 + 32 * j
                                nc.tensor.matmul(
                                    S[:, j * 512:(j + 1) * 512],
                                    khT[ti][po:po + 32, kk * 128:(kk + 1) * 128],
                                    qhT[ti][po:po + 32, jq * 512:(jq + 1) * 512],
                                    start=True, stop=True, tile_position=(po, 0))
                            P = pbuf.tile([128, 1024], BF16, tag="P", name="P")
                            if kk in dve_set:
                                i1 = ipool.tile([128, 1024], I32, tag="i1", name="i1")
                                nc.vector.tensor_scalar(i1[:], S[:], A_H, B1, MULT, ADD)
                                i2 = ipool.tile([128, 1024], I32, tag="i2", name="i2")
                                nc.vector.tensor_scalar_add(i2[:], i1[:], DI)
                                nc.vector.tensor_mul(P[:], i1[:].bitcast(F32),
                                                     i2[:].bitcast(F32))
                            else:
                                nc.scalar.activation(P[:], S[:], EXP)
                            for j in range(2):
                                nc.tensor.matmul(
                                    psO[32 * j:32 * j + 32, :],
                                    vh[kk][:, 32 * (h0 + j):32 * (h0 + j) + 32],
                                    P[:, j * 512:(j + 1) * 512],
                                    start=first, stop=last,
                                    tile_position=(0, 32 * j),
                                    skip_group_check=True)
                            for j in range(2):
                                nc.tensor.matmul(
                                    psO[64 + 32 * j:65 + 32 * j, :],
                                    ones1[:],
                                    P[:, j * 512:(j + 1) * 512],
                                    start=first, stop=last,
                                    tile_position=(0, 64 + 32 * j),
                                    skip_group_check=True)
                        stO = stage.tile([64, 512], F32, tag="stO", name="stO")
                        stL = stage.tile([64, 512], F32, tag="stL", name="stL")
                        nc.vector.tensor_copy(stO[:], psO[0:64, :])
                        nc.vector.tensor_copy(stL[0:1, :], psO[64:65, :])
                        nc.vector.tensor_copy(stL[32:33, :], psO[96:97, :])
                        zi = z_in[jq][pi]
                        nc.sync.dma_start(
                            out=zi[:, 0:64, :].rearrange("r p c -> p r c"),
                            in_=stO[:].rearrange("p (r c) -> p r c", r=NC_CORES))
                        nc.sync.dma_start(
                            out=zi[:, 64:66, :].rearrange("r p c -> p r c"),
                            in_=stL[0:64:32, :].rearrange("p (r c) -> p r c", r=NC_CORES))
                        nc.gpsimd.collective_compute(
                            "ReduceScatter", mybir.AluOpType.add,
                            replica_groups=[list(range(NC_CORES))],
                            ins=[zi.opt()], outs=[z_out[jq][pi].opt()])

                # ---- per-(jq,pi) divide as each RS lands (SBUF-only work) ----
                with tc.tile_pool(name="ep", bufs=1) as ep:
                    osum = [ep.tile([128, 256], F32, tag=f"osum{i}", name=f"osum{i}") for i in range(2)]
                    lsum32 = [ep.tile([128, 256], F32, tag=f"lsum32{i}", name=f"lsum32{i}") for i in range(2)]
                    for i in range(2):
                        nc.gpsimd.memset(lsum32[i][:], 1.0)
                    rl32 = [ep.tile([128, 256], F32, tag=f"rl32{i}", name=f"rl32{i}") for i in range(2)]
                    attnT = [ep.tile([128, 256], BF16, tag=f"attnT{i}", name=f"attnT{i}") for i in range(2)]
                    rl_rep = [ep.tile([128, 256], F32, tag=f"rlrep{i}", name=f"rlrep{i}") for i in range(2)]
                    for jq in range(NJQ):
                        cs = slice(QG * jq, QG * (jq + 1))
                        for pi in range(4):
                            half, i = pi // 2, pi % 2
                            ti, ro = half, 64 * i
                            nc.sync.dma_start(out=osum[ti][ro:ro + 64, cs],
                                              in_=z_out[jq][pi][0:64, :])
                            nc.sync.dma_start(out=lsum32[ti][ro:ro + 64:32, cs],
                                              in_=z_out[jq][pi][64:66, :])
                            nc.vector.reciprocal(rl32[ti][ro:ro + 64, cs],
                                                 lsum32[ti][ro:ro + 64, cs])
                            for j in range(2):
                                po = ro + 32 * j
                                rb1 = ep.tile([1, QG], F32, name="rbt1", tag="rbt1", bufs=2)
                                rb32 = ep.tile([32, QG], F32, name="rbt32", tag="rbt32", bufs=2)
                                nc.gpsimd.tensor_copy(rb1[:], rl32[ti][po:po + 1, cs])
                                nc.gpsimd.partition_broadcast(rb32[:], rb1[0:1, :])
                                nc.gpsimd.tensor_copy(rl_rep[ti][po:po + 32, cs], rb32[:])
                            nc.vector.tensor_mul(attnT[ti][ro:ro + 64, cs],
                                                 osum[ti][ro:ro + 64, cs],
                                                 rl_rep[ti][ro:ro + 64, cs])

                    # ---- tail epilogue: out-projection, bias, transpose, store ----
                    psum_out = pO.tile([128, 512], F32, tag="O", name="psum_out")
                    for jq in range(NJQ):
                        cs = slice(QG * jq, QG * (jq + 1))
                        for dc in range(2):
                            for m in range(2):
                                nc.tensor.matmul(psum_out[:, 256 * dc + QG * jq:
                                                          256 * dc + QG * (jq + 1)],
                                                 wo_r[m][:, dc * 128:(dc + 1) * 128],
                                                 attnT[m][:, cs], start=(m == 0), stop=(m == 1),
                                                 skip_group_check=True)
                    oT = [ep.tile([128, 256], F32, tag=f"oT{i}", name=f"oT{i}") for i in range(2)]
                    out_sb = [ep.tile([128, 256], F32, tag=f"outsb{i}", name=f"outsb{i}") for i in range(2)]
                    for dc in range(2):
                        nc.vector.tensor_scalar_add(oT[dc][:], psum_out[:, 256 * dc:256 * (dc + 1)],
                                                    bo_c[dc])
                    for qc in range(2):
                        qs = slice(qc * 128, (qc + 1) * 128)
                        pt2 = pO.tile([128, 512], F32, tag="O", name="ptout")
                        for dc in range(2):
                            nc.tensor.transpose(pt2[:, dc * 128:(dc + 1) * 128],
                                                oT[dc][:, qs], ident[:])
                        nc.vector.tensor_copy(out_sb[qc][:], pt2[:, 0:256])
                        nc.sync.dma_start(out=t_out[qc * 128:(qc + 1) * 128, :], in_=out_sb[qc][:])

    nc.compile()
    return nc


_NC_CACHE = {}


def _get_nc():
    if "nc" not in _NC_CACHE:
        _NC_CACHE["nc"] = build_nc()
    return _NC_CACHE["nc"]


def run_cores(inputs, trace=False):
    nc = _get_nc()
    full = {k: np.ascontiguousarray(np.asarray(v, dtype=np.float32)) for k, v in inputs.items()}
    in_maps = []
    for c in range(NC_CORES):
        m = dict(full)
        m["key"] = np.ascontiguousarray(full["key"][c * TLOC:(c + 1) * TLOC])
        m["value"] = np.ascontiguousarray(full["value"][c * TLOC:(c + 1) * TLOC])
        in_maps.append(m)
    res = run_bass_kernel_spmd(nc, in_maps, core_ids=list(range(NC_CORES)), trace=trace)
    out = np.empty((TQ, DOUT), dtype=np.float32)
    for r in range(NC_CORES):
        blk = res.results[r]["out"]
        for jq in range(NJQ):
            q0 = QG * (NC_CORES * jq + r)
            out[q0:q0 + QG, :] = blk[QG * jq:QG * (jq + 1), :]
    return out, res


def kernel(**inputs) -> np.ndarray:
    out, _ = run_cores(inputs, trace=False)
    return out


# revision 9
# speedup vs baseline: 1.0197x; 1.0197x over previous
"""Sequence-parallel attention kernel for 8 Trainium2 NeuronCores.

Problem: nn_Attention_v2 — QKV projections + softmax attention + out-proj.
  query [2048,256], key/value [16384,256], weights [256,256], H=8 heads, KD=VD=32.

Sharding: K/V sequence split 8 ways (2048 rows/core); query replicated.
Each core computes, for all 8 heads, the *unnormalized* attention numerator
Onum = exp(S) @ V and denominator l = exp(S) @ 1 over its local K/V chunk
(no max subtraction needed: logits ~ N(0,1), |S| < ~7, exp is safe in fp32).
A ReduceScatter sums (Onum, l) across cores and shards the result by query
columns; each core then divides, applies the output projection for its query
shard, and the host concatenates the 8 shards.

Fast path vs the original version:
- exp is split between the ACT engine (exact, LUT) and the DVE using a
  3-op bit-trick product approx (two phase-shifted Schraudolph factors,
  ~1.5% max rel err) on 5/16 of key chunks -> ~6e-3 output rel err
  (budget 2e-2): i1 = int32(A*s+B); i2 = i1 + dI; P = bits(i1)*bits(i2).
- All matmul operands are bf16 (weights, transposed inputs, projected
  heads, P): 1-pass PE streams, cheap LDWEIGHTS, and column packing works
  (f32r matmuls require dst partition 0; bf16 doesn't). PSUM accumulation
  stays fp32, so only operand quantization noise (~0.2%) is added.
- AV + softmax-denominator matmuls pack 4-wide per head-pair: numerators
  at psum partitions 0/32 (M=32 each), denominators via ones-column
  matmuls at partitions 64/96 (M=1), all streaming concurrently on
  separate column groups of the PE array.
- Prologue (transpose + project k/q/v) is emitted interleaved with the
  first attention block so the shared PSUM ring never serializes
  attention behind the whole prologue.
"""
import sys

sys.path.insert(0, "/opt/trn_rl_repo")

import numpy as np

import concourse.bass as bass  # noqa: F401  (import order matters)
from concourse import bacc
import concourse.mybir as mybir
from concourse.bass_utils import run_bass_kernel_spmd
from concourse.tile import TileContext
from concourse.masks import make_identity

F32 = mybir.dt.float32
BF16 = mybir.dt.bfloat16
I32 = mybir.dt.int32
EXP = mybir.ActivationFunctionType.Exp
MULT = mybir.AluOpType.mult
ADD = mybir.AluOpType.add

NC_CORES = 8
TQ, T, D = 2048, 16384, 256
H, KD, VD, DOUT = 8, 32, 32, 256
HD = H * KD  # 256
TLOC = T // NC_CORES          # 2048 local K/V rows
NKT = TLOC // 128             # 16 k-chunks
NJQ = TQ // 512               # 4 q-column chunks of 512
QG = 64                       # q columns per rank-group in the RS layout
SCALE = float(1.0 / np.sqrt(KD))

# DVE 3-op exp approximation constants (fitted; ~1.54% max rel err)
A_H = float(np.float32(2**22 * 1.4426950408889634))
B1 = float(np.float32(1066999681.734))
DI = -4193778


def dve_kk_set(b):
    """Which k-chunks of block b the DVE computes exp for (rest on ACT).
    Early blocks are ACT-only: the DVE is busy with prologue transposes."""
    if b < 2:
        return frozenset()
    if b == 2:
        return frozenset({5, 11})
    return frozenset({1, 4, 7, 10, 13})


def build_nc():
    nc = bacc.Bacc("TRN2", target_bir_lowering=False)

    t_query = nc.dram_tensor("query", [TQ, D], F32, kind="ExternalInput")
    t_key = nc.dram_tensor("key", [TLOC, D], F32, kind="ExternalInput")
    t_value = nc.dram_tensor("value", [TLOC, D], F32, kind="ExternalInput")
    t_wq = nc.dram_tensor("wq", [D, HD], F32, kind="ExternalInput")
    t_wk = nc.dram_tensor("wk", [D, HD], F32, kind="ExternalInput")
    t_wv = nc.dram_tensor("wv", [D, HD], F32, kind="ExternalInput")
    t_wo = nc.dram_tensor("wo", [HD, DOUT], F32, kind="ExternalInput")
    t_bq = nc.dram_tensor("bq", [HD], F32, kind="ExternalInput")
    t_bk = nc.dram_tensor("bk", [HD], F32, kind="ExternalInput")
    t_bv = nc.dram_tensor("bv", [HD], F32, kind="ExternalInput")
    t_bo = nc.dram_tensor("bo", [DOUT], F32, kind="ExternalInput")
    t_out = nc.dram_tensor("out", [TQ // NC_CORES, DOUT], F32, kind="ExternalOutput")

    with TileContext(nc) as tc:
        with tc.tile_pool(name="const", bufs=1) as constp, \
             tc.tile_pool(name="persist", bufs=1) as persist, \
             tc.tile_pool(name="dram", bufs=1, space="DRAM") as dramp:

            ident = constp.tile([128, 128], F32)
            make_identity(nc, ident[:])
            ones_f = constp.tile([128, 1], F32)
            nc.gpsimd.memset(ones_f[:], 1.0)
            ones1 = constp.tile([128, 1], BF16)
            nc.vector.tensor_copy(ones1[:], ones_f[:])

            # projected, transposed activations (feature rows on partitions)
            qhT = [persist.tile([128, TQ], BF16, tag=f"qhT{m}", name=f"qhT{m}") for m in range(2)]
            khT = [persist.tile([128, TLOC], BF16, tag=f"khT{m}", name=f"khT{m}") for m in range(2)]
            vh = [persist.tile([128, 256], BF16, tag=f"vh{t}", name=f"vh{t}") for t in range(NKT)]

            with tc.tile_pool(name="tin", bufs=6) as tin, \
                 tc.tile_pool(name="tT", bufs=1) as tTp, \
                 tc.tile_pool(name="pS", bufs=3, space="PSUM") as pS, \
                 tc.tile_pool(name="pO", bufs=2, space="PSUM") as pO, \
                 tc.tile_pool(name="pbuf", bufs=10) as pbuf, \
                 tc.tile_pool(name="ipool", bufs=2) as ipool, \
                 tc.tile_pool(name="wstage", bufs=2) as wstage, \
                 tc.tile_pool(name="stage", bufs=2) as stage:

                def ps_tile():
                    # shared 3-deep PSUM ring: prologue transposes/projections
                    # and the attention S tiles all draw [128,1024] slots
                    return pS.tile([128, 1024], F32, tag="S", name="ps")

                # ---- weights + biases to SBUF (bf16; wq,bq pre-scaled). ----
                # One DMA per weight: [256,256] -> [128, 512] with D-chunk a in
                # cols 256a..; lhsT slice for (dc, m) = [:, 256*dc+128*m :+128].
                wcomb = {}

                def load_w(tdram, key, scale_mul):
                    raw = wstage.tile([128, 512], F32, tag="wraw", name="wraw")
                    nc.sync.dma_start(
                        out=raw[:].rearrange("p (a d) -> p a d", a=2),
                        in_=tdram[:].rearrange("(a p) d -> p a d", a=2))
                    wt = persist.tile([128, 512], BF16, tag=f"w_{key}", name=f"w_{key}")
                    if scale_mul is not None:
                        nc.vector.tensor_scalar_mul(wt[:], raw[:], scale_mul)
                    else:
                        nc.vector.tensor_copy(wt[:], raw[:])
                    wcomb[key] = wt
                    return [wt[:, 256 * dc:256 * (dc + 1)] for dc in range(2)]

                def load_b(tdram, dst, scale_mul, key):
                    braw = wstage.tile([128, 2], F32, tag="braw", name="braw")
                    nc.sync.dma_start(out=braw[:],
                                      in_=tdram[:].rearrange("(a p) -> p a", a=2))
                    bt = persist.tile([128, 2], F32, tag=f"b_{key}", name=f"b_{key}")
                    if scale_mul is not None:
                        nc.vector.tensor_scalar_mul(bt[:], braw[:], scale_mul)
                    else:
                        nc.vector.tensor_copy(bt[:], braw[:])
                    for m in range(2):
                        dst[m] = bt[:, m:m + 1]

                # transposed raw inputs in bf16, token-tile-major with the two
                # D-chunks of each 128-token tile adjacent:
                # cols [256*i + 128*dc : ...+128] = tile i, D rows 128dc..
                qT = tTp.tile([128, 2 * TQ], BF16, tag="qT", name="qT")
                kT = tTp.tile([128, 2 * TLOC], BF16, tag="kT", name="kT")
                vT = tTp.tile([128, 2 * TLOC], BF16, tag="vT", name="vT")

                def load_tile(tdram, dst, i, tag):
                    """DMA 128 rows of [tok,256], transpose both 128-col halves
                    into one psum tile, single contiguous copy out (bf16)."""
                    raw = tin.tile([128, 256], F32, tag=f"in_{tag}", name=f"in_{tag}")
                    nc.sync.dma_start(out=raw[:], in_=tdram[i * 128:(i + 1) * 128, :])
                    pt = ps_tile()
                    for m in range(2):
                        nc.tensor.transpose(pt[:, m * 128:(m + 1) * 128],
                                            raw[:, m * 128:(m + 1) * 128], ident[:])
                    nc.vector.tensor_copy(dst[:, 256 * i:256 * (i + 1)], pt[:, 0:256])

                def proj_rhs(src, ntok, j, dc):
                    # tokens 512j..512(j+1), D-chunk dc: 4 strided 128-blocks
                    v = src[:].rearrange("p (i b) -> p i b", b=256)
                    return v[:, 4 * j:4 * (j + 1), 128 * dc:128 * (dc + 1)]

                def kchunk(j):
                    for i in range(4 * j, 4 * j + 4):
                        load_tile(t_key, kT, i, "kT")
                    for m in range(2):
                        pp = ps_tile()
                        for dc in range(2):
                            nc.tensor.matmul(pp[:, 0:512],
                                             wk_r[dc][:, m * 128:(m + 1) * 128],
                                             proj_rhs(kT, TLOC, j, dc),
                                             start=(dc == 0), stop=(dc == 1))
                        nc.vector.tensor_scalar_add(khT[m][:, j * 512:(j + 1) * 512],
                                                    pp[:, 0:512], bk_c[m])

                def qchunk(j):
                    for i in range(4 * j, 4 * j + 4):
                        load_tile(t_query, qT, i, "qT")
                    for m in range(2):
                        pp = ps_tile()
                        for dc in range(2):
                            nc.tensor.matmul(pp[:, 0:512],
                                             wq_r[dc][:, m * 128:(m + 1) * 128],
                                             proj_rhs(qT, TQ, j, dc),
                                             start=(dc == 0), stop=(dc == 1))
                        nc.vector.tensor_scalar_add(qhT[m][:, j * 512:(j + 1) * 512],
                                                    pp[:, 0:512], bq_c[m])

                def vchunk(j):
                    for t in range(4 * j, 4 * j + 4):
                        load_tile(t_value, vT, t, "vT")
                        pp = ps_tile()
                        for dc in range(2):
                            nc.tensor.matmul(pp[:, 0:256],
                                             vT[:, 256 * t + 128 * dc:256 * t + 128 * (dc + 1)],
                                             wv_r[dc][:], start=(dc == 0), stop=(dc == 1))
                        nc.vector.tensor_add(vh[t][:], pp[:, 0:256], bv_rep[:])

                # ---- prologue head: everything block 0 needs to start ----
                wk_r = load_w(t_wk, "wk", None)
                wq_r = load_w(t_wq, "wq", SCALE)
                bq_c, bk_c, bo_c = [None, None], [None, None], [None, None]
                load_b(t_bk, bk_c, None, "bk")
                load_b(t_bq, bq_c, SCALE, "bq")
                wv_r = load_w(t_wv, "wv", None)
                bv_row = persist.tile([1, 256], F32)
                nc.sync.dma_start(out=bv_row[:], in_=t_bv[:].rearrange("(a d) -> a d", a=1))
                bv_rep = persist.tile([128, 256], F32)
                nc.gpsimd.partition_broadcast(bv_rep[:], bv_row[0:1, :])
                kchunk(0)
                qchunk(0)
                vchunk(0)

                # remaining prologue, emitted interleaved into the first blocks
                prologue = []
                for j in range(1, 4):
                    prologue.append(lambda j=j: kchunk(j))
                    prologue.append(lambda j=j: vchunk(j))

                def late_prologue(b):
                    if b == 1:
                        wo_r.extend(load_w(t_wo, "wo", None))
                        load_b(t_bo, bo_c, None, "bo")
                        qchunk(1)
                    elif b == 2:
                        qchunk(2)
                    elif b == 3:
                        qchunk(3)

                wo_r = []

                # ---- main attention loop + per-(jq, head-pair) ReduceScatter ----
                z_in = [[dramp.tile([NC_CORES, 66, QG], F32, tag=f"zin{j}_{p}", name=f"zin{j}_{p}")
                         for p in range(4)] for j in range(NJQ)]
                z_out = [[dramp.tile([66, QG], F32, tag=f"zout{j}_{p}", name=f"zout{j}_{p}")
                          for p in range(4)] for j in range(NJQ)]

                for jq in range(NJQ):
                    for pi in range(4):
                        b = 4 * jq + pi
                        dve_set = dve_kk_set(b)
                        h0 = 2 * pi
                        ti = h0 // 4
                        po0 = 32 * (h0 % 4)
                        # psO: head h0 rows at partitions 0-31, h1 at 32-63,
                        # softmax denominators at partitions 64 and 96.
                        psO = pO.tile([128, 512], F32, tag="O", name="psO")
                        for kk in range(NKT):
                            if b == 0 and kk in (4, 8, 12):
                                prologue.pop(0)()   # kchunk(kk//4)
                                prologue.pop(0)()   # vchunk(kk//4)
                            first, last = kk == 0, kk == NKT - 1
                            S = ps_tile()
                            for j in range(2):
                                po = po0 + 32 * j
                                nc.tensor.matmul(
                                    S[:, j * 512:(j + 1) * 512],
                                    khT[ti][po:po + 32, kk * 128:(kk + 1) * 128],
                                    qhT[ti][po:po + 32, jq * 512:(jq + 1) * 512],
                                    start=True, stop=True, tile_position=(po, 0))
                            P = pbuf.tile([128, 1024], BF16, tag="P", name="P")
                            if kk in dve_set:
                                i1 = ipool.tile([128, 1024], I32, tag="i1", name="i1")
                                nc.vector.tensor_scalar(i1[:], S[:], A_H, B1, MULT, ADD)
                                i2 = ipool.tile([128, 1024], I32, tag="i2", name="i2")
                                nc.vector.tensor_scalar_add(i2[:], i1[:], DI)
                                nc.vector.tensor_mul(P[:], i1[:].bitcast(F32),
                                                     i2[:].bitcast(F32))
                            else:
                                nc.scalar.activation(P[:], S[:], EXP)
                            for j in range(2):
                                nc.tensor.matmul(
                                    psO[32 * j:32 * j + 32, :],
                                    vh[kk][:, 32 * (h0 + j):32 * (h0 + j) + 32],
                                    P[:, j * 512:(j + 1) * 512],
                                    start=first, stop=last,
                                    tile_position=(0, 32 * j),
                                    skip_group_check=True)
                            for j in range(2):
                                nc.tensor.matmul(
                                    psO[64 + 32 * j:65 + 32 * j, :],
                                    ones1[:],
                                    P[:, j * 512:(j + 1) * 512],
                                    start=first, stop=last,
                                    tile_position=(0, 64 + 32 * j),
                                    skip_group_check=True)
                        stO = stage.tile([64, 512], F32, tag="stO", name="stO")
                        stL = stage.tile([64, 512], F32, tag="stL", name="stL")
                        nc.vector.tensor_copy(stO[:], psO[0:64, :])
                        nc.vector.tensor_copy(stL[0:1, :], psO[64:65, :])
                        nc.vector.tensor_copy(stL[32:33, :], psO[96:97, :])
                        zi = z_in[jq][pi]
                        nc.sync.dma_start(
                            out=zi[:, 0:64, :].rearrange("r p c -> p r c"),
                            in_=stO[:].rearrange("p (r c) -> p r c", r=NC_CORES))
                        nc.sync.dma_start(
                            out=zi[:, 64:66, :].rearrange("r p c -> p r c"),
                            in_=stL[0:64:32, :].rearrange("p (r c) -> p r c", r=NC_CORES))
                        nc.gpsimd.collective_compute(
                            "ReduceScatter", mybir.AluOpType.add,
                            replica_groups=[list(range(NC_CORES))],
                            ins=[zi.opt()], outs=[z_out[jq][pi].opt()])
                        late_prologue(b)

                # ---- per-(jq,pi) divide as each RS lands (SBUF-only work) ----
                with tc.tile_pool(name="ep", bufs=1) as ep:
                    osum = [ep.tile([128, 256], F32, tag=f"osum{i}", name=f"osum{i}") for i in range(2)]
                    lsum32 = [ep.tile([128, 256], F32, tag=f"lsum32{i}", name=f"lsum32{i}") for i in range(2)]
                    for i in range(2):
                        nc.gpsimd.memset(lsum32[i][:], 1.0)
                    rl32 = [ep.tile([128, 256], F32, tag=f"rl32{i}", name=f"rl32{i}") for i in range(2)]
                    attnT = [ep.tile([128, 256], BF16, tag=f"attnT{i}", name=f"attnT{i}") for i in range(2)]
                    rl_rep = [ep.tile([128, 256], F32, tag=f"rlrep{i}", name=f"rlrep{i}") for i in range(2)]
                    for jq in range(NJQ):
                        cs = slice(QG * jq, QG * (jq + 1))
                        for pi in range(4):
                            half, i = pi // 2, pi % 2
                            ti, ro = half, 64 * i
                            nc.sync.dma_start(out=osum[ti][ro:ro + 64, cs],
                                              in_=z_out[jq][pi][0:64, :])
                            nc.sync.dma_start(out=lsum32[ti][ro:ro + 64:32, cs],
                                              in_=z_out[jq][pi][64:66, :])
                            nc.vector.reciprocal(rl32[ti][ro:ro + 64, cs],
                                                 lsum32[ti][ro:ro + 64, cs])
                            for j in range(2):
                                po = ro + 32 * j
                                rb1 = ep.tile([1, QG], F32, name="rbt1", tag="rbt1", bufs=2)
                                rb32 = ep.tile([32, QG], F32, name="rbt32", tag="rbt32", bufs=2)
                                nc.gpsimd.tensor_copy(rb1[:], rl32[ti][po:po + 1, cs])
                                nc.gpsimd.partition_broadcast(rb32[:], rb1[0:1, :])
                                nc.gpsimd.tensor_copy(rl_rep[ti][po:po + 32, cs], rb32[:])
                            nc.vector.tensor_mul(attnT[ti][ro:ro + 64, cs],
                                                 osum[ti][ro:ro + 64, cs],
                                                 rl_rep[ti][ro:ro + 64, cs])

                    # ---- tail epilogue: out-projection, bias, transpose, store ----
                    psum_out = pO.tile([128, 512], F32, tag="O", name="psum_out")
                    for jq in range(NJQ):
                        cs = slice(QG * jq, QG * (jq + 1))
                        for dc in range(2):
                            for m in range(2):
                                nc.tensor.matmul(psum_out[:, 256 * dc + QG * jq:
                                                          256 * dc + QG * (jq + 1)],
                                                 wo_r[m][:, dc * 128:(dc + 1) * 128],
                                                 attnT[m][:, cs], start=(m == 0), stop=(m == 1),
                                                 skip_group_check=True)
                    oT = [ep.tile([128, 256], F32, tag=f"oT{i}", name=f"oT{i}") for i in range(2)]
                    out_sb = [ep.tile([128, 256], F32, tag=f"outsb{i}", name=f"outsb{i}") for i in range(2)]
                    for dc in range(2):
                        nc.vector.tensor_scalar_add(oT[dc][:], psum_out[:, 256 * dc:256 * (dc + 1)],
                                                    bo_c[dc])
                    for qc in range(2):
                        qs = slice(qc * 128, (qc + 1) * 128)
                        pt2 = pO.tile([128, 512], F32, tag="O", name="ptout")
                        for dc in range(2):
                            nc.tensor.transpose(pt2[:, dc * 128:(dc + 1) * 128],
                                                oT[dc][:, qs], ident[:])
                        nc.vector.tensor_copy(out_sb[qc][:], pt2[:, 0:256])
                        nc.sync.dma_start(out=t_out[qc * 128:(qc + 1) * 128, :], in_=out_sb[qc][:])

    nc.compile()
    return nc


_NC_CACHE = {}


def _get_nc():
    if "nc" not in _NC_CACHE:
        _NC_CACHE["nc"] = build_nc()
    return _NC_CACHE["nc"]


def run_cores(inputs, trace=False):
    nc = _get_nc()
    full = {k: np.ascontiguousarray(np.asarray(v, dtype=np.float32)) for k, v in inputs.items()}
    in_maps = []
    for c in range(NC_CORES):
        m = dict(full)
        m["key"] = np.ascontiguousarray(full["key"][c * TLOC:(c + 1) * TLOC])
        m["value"] = np.ascontiguousarray(full["value"][c * TLOC:(c + 1) * TLOC])
        in_maps.append(m)
    res = run_bass_kernel_spmd(nc, in_maps, core_ids=list(range(NC_CORES)), trace=trace)
    out = np.empty((TQ, DOUT), dtype=np.float32)
    for r in range(NC_CORES):
        blk = res.results[r]["out"]
        for jq in range(NJQ):
            q0 = QG * (NC_CORES * jq + r)
            out[q0:q0 + QG, :] = blk[QG * jq:QG * (jq + 1), :]
    return out, res


def kernel(**inputs) -> np.ndarray:
    out, _ = run_cores(inputs, trace=False)
    return out


# revision 10
# speedup vs baseline: 1.0672x; 1.0465x over previous
"""Sequence-parallel attention kernel for 8 Trainium2 NeuronCores.

Problem: nn_Attention_v2 — QKV projections + softmax attention + out-proj.
  query [2048,256], key/value [16384,256], weights [256,256], H=8 heads, KD=VD=32.

Sharding: K/V sequence split 8 ways (2048 rows/core); query replicated.
Each core computes, for all 8 heads, the *unnormalized* attention numerator
Onum = exp(S) @ V and denominator l = exp(S) @ 1 over its local K/V chunk
(no max subtraction needed: logits ~ N(0,1), |S| < ~7, exp is safe in fp32).
A ReduceScatter sums (Onum, l) across cores and shards the result by query
columns; each core then divides, applies the output projection for its query
shard, and the host concatenates the 8 shards.

Fast path vs the original version:
- exp is split between the ACT engine (exact, LUT) and the DVE using a
  3-op bit-trick product approx (two phase-shifted Schraudolph factors,
  ~1.5% max rel err) on 5/16 of key chunks -> ~6e-3 output rel err
  (budget 2e-2): i1 = int32(A*s+B); i2 = i1 + dI; P = bits(i1)*bits(i2).
- All matmul operands are bf16 (weights, transposed inputs, projected
  heads, P): 1-pass PE streams, cheap LDWEIGHTS, and column packing works
  (f32r matmuls require dst partition 0; bf16 doesn't). PSUM accumulation
  stays fp32, so only operand quantization noise (~0.2%) is added.
- AV + softmax-denominator matmuls pack 4-wide per head-pair: numerators
  at psum partitions 0/32 (M=32 each), denominators via ones-column
  matmuls at partitions 64/96 (M=1), all streaming concurrently on
  separate column groups of the PE array.
- Prologue (transpose + project k/q/v) is emitted interleaved with the
  first attention block so the shared PSUM ring never serializes
  attention behind the whole prologue.
"""
import sys

sys.path.insert(0, "/opt/trn_rl_repo")

import numpy as np

import concourse.bass as bass  # noqa: F401  (import order matters)
from concourse import bacc
import concourse.mybir as mybir
from concourse.bass_utils import run_bass_kernel_spmd
from concourse.tile import TileContext
from concourse.masks import make_identity

F32 = mybir.dt.float32
F32R = mybir.dt.float32r
BF16 = mybir.dt.bfloat16
I32 = mybir.dt.int32
EXP = mybir.ActivationFunctionType.Exp
MULT = mybir.AluOpType.mult
ADD = mybir.AluOpType.add

NC_CORES = 8
TQ, T, D = 2048, 16384, 256
H, KD, VD, DOUT = 8, 32, 32, 256
HD = H * KD  # 256
TLOC = T // NC_CORES          # 2048 local K/V rows
NKT = TLOC // 128             # 16 k-chunks
NJQ = TQ // 512               # 4 q-column chunks of 512
QG = 64                       # q columns per rank-group in the RS layout
SCALE = float(1.0 / np.sqrt(KD))

# DVE 3-op exp approximation constants (fitted; ~1.54% max rel err)
A_H = float(np.float32(2**22 * 1.4426950408889634))
B1 = float(np.float32(1066999681.734))
DI = -4193778


def dve_kk_set(b):
    """Which k-chunks of block b the DVE computes exp for (rest on ACT).
    Early blocks are ACT-only: the DVE is busy with prologue transposes."""
    if b < 2:
        return frozenset()
    if b == 2:
        return frozenset({5, 11})
    return frozenset({1, 4, 7, 10, 13})


def build_nc():
    nc = bacc.Bacc("TRN2", target_bir_lowering=False)

    t_query = nc.dram_tensor("query", [TQ, D], F32, kind="ExternalInput")
    t_key = nc.dram_tensor("key", [TLOC, D], F32, kind="ExternalInput")
    t_value = nc.dram_tensor("value", [TLOC, D], F32, kind="ExternalInput")
    t_wq = nc.dram_tensor("wq", [D, HD], F32, kind="ExternalInput")
    t_wk = nc.dram_tensor("wk", [D, HD], F32, kind="ExternalInput")
    t_wv = nc.dram_tensor("wv", [D, HD], F32, kind="ExternalInput")
    t_wo = nc.dram_tensor("wo", [HD, DOUT], F32, kind="ExternalInput")
    t_bq = nc.dram_tensor("bq", [HD], F32, kind="ExternalInput")
    t_bk = nc.dram_tensor("bk", [HD], F32, kind="ExternalInput")
    t_bv = nc.dram_tensor("bv", [HD], F32, kind="ExternalInput")
    t_bo = nc.dram_tensor("bo", [DOUT], F32, kind="ExternalInput")
    t_out = nc.dram_tensor("out", [TQ // NC_CORES, DOUT], F32, kind="ExternalOutput")

    with TileContext(nc) as tc:
        with tc.tile_pool(name="const", bufs=1) as constp, \
             tc.tile_pool(name="persist", bufs=1) as persist, \
             tc.tile_pool(name="dram", bufs=1, space="DRAM") as dramp:

            ident = constp.tile([128, 128], F32)
            make_identity(nc, ident[:])
            ones_f = constp.tile([128, 1], F32)
            nc.gpsimd.memset(ones_f[:], 1.0)
            ones1 = constp.tile([128, 1], BF16)
            nc.vector.tensor_copy(ones1[:], ones_f[:])

            # projected, transposed activations (feature rows on partitions)
            qhT = [persist.tile([128, TQ], BF16, tag=f"qhT{m}", name=f"qhT{m}") for m in range(2)]
            khT = [persist.tile([128, TLOC], BF16, tag=f"khT{m}", name=f"khT{m}") for m in range(2)]
            vh = [persist.tile([128, 256], BF16, tag=f"vh{t}", name=f"vh{t}") for t in range(NKT)]

            with tc.tile_pool(name="tin", bufs=6) as tin, \
                 tc.tile_pool(name="tT", bufs=1) as tTp, \
                 tc.tile_pool(name="pS", bufs=3, space="PSUM") as pS, \
                 tc.tile_pool(name="pO", bufs=2, space="PSUM") as pO, \
                 tc.tile_pool(name="pbuf", bufs=10) as pbuf, \
                 tc.tile_pool(name="ipool", bufs=2) as ipool, \
                 tc.tile_pool(name="wstage", bufs=2) as wstage, \
                 tc.tile_pool(name="stage", bufs=2) as stage:

                def ps_tile():
                    # shared 3-deep PSUM ring: prologue transposes/projections
                    # and the attention S tiles all draw [128,1024] slots
                    return pS.tile([128, 1024], F32, tag="S", name="ps")

                # ---- weights + biases to SBUF (bf16; wq,bq pre-scaled). ----
                # One DMA per weight: [256,256] -> [128, 512] with D-chunk a in
                # cols 256a..; lhsT slice for (dc, m) = [:, 256*dc+128*m :+128].
                wcomb = {}

                def load_w(tdram, key, scale_mul, wdt=F32R):
                    raw = wstage.tile([128, 512], F32, tag="wraw", name="wraw")
                    nc.sync.dma_start(
                        out=raw[:].rearrange("p (a d) -> p a d", a=2),
                        in_=tdram[:].rearrange("(a p) d -> p a d", a=2))
                    wt = persist.tile([128, 512], wdt, tag=f"w_{key}", name=f"w_{key}")
                    if scale_mul is not None:
                        nc.vector.tensor_scalar_mul(wt[:], raw[:], scale_mul)
                    else:
                        nc.vector.tensor_copy(wt[:], raw[:])
                    wcomb[key] = wt
                    return [wt[:, 256 * dc:256 * (dc + 1)] for dc in range(2)]

                def load_b(tdram, dst, scale_mul, key):
                    braw = wstage.tile([128, 2], F32, tag="braw", name="braw")
                    nc.sync.dma_start(out=braw[:],
                                      in_=tdram[:].rearrange("(a p) -> p a", a=2))
                    bt = persist.tile([128, 2], F32, tag=f"b_{key}", name=f"b_{key}")
                    if scale_mul is not None:
                        nc.vector.tensor_scalar_mul(bt[:], braw[:], scale_mul)
                    else:
                        nc.vector.tensor_copy(bt[:], braw[:])
                    for m in range(2):
                        dst[m] = bt[:, m:m + 1]

                # transposed raw inputs in bf16, token-tile-major with the two
                # D-chunks of each 128-token tile adjacent:
                # cols [256*i + 128*dc : ...+128] = tile i, D rows 128dc..
                qT = tTp.tile([128, 2 * TQ], F32R, tag="qT", name="qT")
                kT = tTp.tile([128, 2 * TLOC], F32R, tag="kT", name="kT")
                vT = tTp.tile([128, 2 * TLOC], F32R, tag="vT", name="vT")

                def load_tile(tdram, dst, i, tag):
                    """DMA 128 rows of [tok,256], transpose both 128-col halves
                    into one psum tile, single contiguous copy out (bf16)."""
                    raw = tin.tile([128, 256], F32, tag=f"in_{tag}", name=f"in_{tag}")
                    nc.sync.dma_start(out=raw[:], in_=tdram[i * 128:(i + 1) * 128, :])
                    pt = ps_tile()
                    for m in range(2):
                        nc.tensor.transpose(pt[:, m * 128:(m + 1) * 128],
                                            raw[:, m * 128:(m + 1) * 128], ident[:])
                    nc.vector.tensor_copy(dst[:, 256 * i:256 * (i + 1)], pt[:, 0:256])

                def proj_rhs(src, ntok, j, dc):
                    # tokens 512j..512(j+1), D-chunk dc: 4 strided 128-blocks
                    v = src[:].rearrange("p (i b) -> p i b", b=256)
                    return v[:, 4 * j:4 * (j + 1), 128 * dc:128 * (dc + 1)]

                def kchunk(j):
                    for i in range(4 * j, 4 * j + 4):
                        load_tile(t_key, kT, i, "kT")
                    for m in range(2):
                        pp = ps_tile()
                        for dc in range(2):
                            nc.tensor.matmul(pp[:, 0:512],
                                             wk_r[dc][:, m * 128:(m + 1) * 128],
                                             proj_rhs(kT, TLOC, j, dc),
                                             start=(dc == 0), stop=(dc == 1))
                        nc.vector.tensor_scalar_add(khT[m][:, j * 512:(j + 1) * 512],
                                                    pp[:, 0:512], bk_c[m])

                def qchunk(j):
                    for i in range(4 * j, 4 * j + 4):
                        load_tile(t_query, qT, i, "qT")
                    for m in range(2):
                        pp = ps_tile()
                        for dc in range(2):
                            nc.tensor.matmul(pp[:, 0:512],
                                             wq_r[dc][:, m * 128:(m + 1) * 128],
                                             proj_rhs(qT, TQ, j, dc),
                                             start=(dc == 0), stop=(dc == 1))
                        nc.vector.tensor_scalar_add(qhT[m][:, j * 512:(j + 1) * 512],
                                                    pp[:, 0:512], bq_c[m])

                def vchunk(j):
                    for t in range(4 * j, 4 * j + 4):
                        load_tile(t_value, vT, t, "vT")
                        pp = ps_tile()
                        for dc in range(2):
                            nc.tensor.matmul(pp[:, 0:256],
                                             vT[:, 256 * t + 128 * dc:256 * t + 128 * (dc + 1)],
                                             wv_r[dc][:], start=(dc == 0), stop=(dc == 1))
                        nc.vector.tensor_add(vh[t][:], pp[:, 0:256], bv_rep[:])

                # ---- prologue head: everything block 0 needs to start ----
                wk_r = load_w(t_wk, "wk", None)
                wq_r = load_w(t_wq, "wq", SCALE)
                bq_c, bk_c, bo_c = [None, None], [None, None], [None, None]
                load_b(t_bk, bk_c, None, "bk")
                load_b(t_bq, bq_c, SCALE, "bq")
                wv_r = load_w(t_wv, "wv", None)
                bv_row = persist.tile([1, 256], F32)
                nc.sync.dma_start(out=bv_row[:], in_=t_bv[:].rearrange("(a d) -> a d", a=1))
                bv_rep = persist.tile([128, 256], F32)
                nc.gpsimd.partition_broadcast(bv_rep[:], bv_row[0:1, :])
                kchunk(0)
                qchunk(0)
                vchunk(0)

                # remaining prologue, emitted interleaved into the first blocks
                prologue = []
                for j in range(1, 4):
                    prologue.append(lambda j=j: kchunk(j))
                    prologue.append(lambda j=j: vchunk(j))

                def late_prologue(b):
                    if b == 1:
                        wo_r.extend(load_w(t_wo, "wo", None, BF16))
                        load_b(t_bo, bo_c, None, "bo")
                        qchunk(1)
                    elif b == 2:
                        qchunk(2)
                    elif b == 3:
                        qchunk(3)

                wo_r = []

                # ---- main attention loop + per-(jq, head-pair) ReduceScatter ----
                z_in = [[dramp.tile([NC_CORES, 66, QG], F32, tag=f"zin{j}_{p}", name=f"zin{j}_{p}")
                         for p in range(4)] for j in range(NJQ)]
                z_out = [[dramp.tile([66, QG], F32, tag=f"zout{j}_{p}", name=f"zout{j}_{p}")
                          for p in range(4)] for j in range(NJQ)]

                for jq in range(NJQ):
                    for pi in range(4):
                        b = 4 * jq + pi
                        dve_set = dve_kk_set(b)
                        h0 = 2 * pi
                        ti = h0 // 4
                        po0 = 32 * (h0 % 4)
                        # psO: head h0 rows at partitions 0-31, h1 at 32-63,
                        # softmax denominators at partitions 64 and 96.
                        psO = pO.tile([128, 512], F32, tag="O", name="psO")
                        Pq = []

                        def av_group(kk, P):
                            first, last = kk == 0, kk == NKT - 1
                            for j in range(2):
                                nc.tensor.matmul(
                                    psO[32 * j:32 * j + 32, :],
                                    vh[kk][:, 32 * (h0 + j):32 * (h0 + j) + 32],
                                    P[:, j * 512:(j + 1) * 512],
                                    start=first, stop=last,
                                    tile_position=(0, 32 * j),
                                    skip_group_check=True)
                            for j in range(2):
                                nc.tensor.matmul(
                                    psO[64 + 32 * j:65 + 32 * j, :],
                                    ones1[:],
                                    P[:, j * 512:(j + 1) * 512],
                                    start=first, stop=last,
                                    tile_position=(0, 64 + 32 * j),
                                    skip_group_check=True)

                        for kk in range(NKT):
                            if b == 0 and kk in (4, 8, 12):
                                prologue.pop(0)()   # kchunk(kk//4)
                                prologue.pop(0)()   # vchunk(kk//4)
                            S = ps_tile()
                            for j in range(2):
                                po = po0 + 32 * j
                                nc.tensor.matmul(
                                    S[:, j * 512:(j + 1) * 512],
                                    khT[ti][po:po + 32, kk * 128:(kk + 1) * 128],
                                    qhT[ti][po:po + 32, jq * 512:(jq + 1) * 512],
                                    start=True, stop=True, tile_position=(po, 0))
                            P = pbuf.tile([128, 1024], BF16, tag="P", name="P")
                            if kk in dve_set:
                                i1 = ipool.tile([128, 1024], I32, tag="i1", name="i1")
                                nc.vector.tensor_scalar(i1[:], S[:], A_H, B1, MULT, ADD)
                                i2 = ipool.tile([128, 1024], I32, tag="i2", name="i2")
                                nc.vector.tensor_scalar_add(i2[:], i1[:], DI)
                                nc.vector.tensor_mul(P[:], i1[:].bitcast(F32),
                                                     i2[:].bitcast(F32))
                            else:
                                nc.scalar.activation(P[:], S[:], EXP)
                            Pq.append((kk, P))
                            if len(Pq) > 2:
                                av_group(*Pq.pop(0))
                        while Pq:
                            av_group(*Pq.pop(0))
                        stO = stage.tile([64, 512], F32, tag="stO", name="stO")
                        stL = stage.tile([64, 512], F32, tag="stL", name="stL")
                        nc.vector.tensor_copy(stO[:], psO[0:64, :])
                        nc.vector.tensor_copy(stL[0:1, :], psO[64:65, :])
                        nc.vector.tensor_copy(stL[32:33, :], psO[96:97, :])
                        zi = z_in[jq][pi]
                        nc.sync.dma_start(
                            out=zi[:, 0:64, :].rearrange("r p c -> p r c"),
                            in_=stO[:].rearrange("p (r c) -> p r c", r=NC_CORES))
                        nc.sync.dma_start(
                            out=zi[:, 64:66, :].rearrange("r p c -> p r c"),
                            in_=stL[0:64:32, :].rearrange("p (r c) -> p r c", r=NC_CORES))
                        nc.gpsimd.collective_compute(
                            "ReduceScatter", mybir.AluOpType.add,
                            replica_groups=[list(range(NC_CORES))],
                            ins=[zi.opt()], outs=[z_out[jq][pi].opt()])
                        late_prologue(b)

                # ---- per-(jq,pi) divide as each RS lands (SBUF-only work) ----
                with tc.tile_pool(name="ep", bufs=1) as ep:
                    osum = [ep.tile([128, 256], F32, tag=f"osum{i}", name=f"osum{i}") for i in range(2)]
                    lsum32 = [ep.tile([128, 256], F32, tag=f"lsum32{i}", name=f"lsum32{i}") for i in range(2)]
                    for i in range(2):
                        nc.gpsimd.memset(lsum32[i][:], 1.0)
                    rl32 = [ep.tile([128, 256], F32, tag=f"rl32{i}", name=f"rl32{i}") for i in range(2)]
                    attnT = [ep.tile([128, 256], BF16, tag=f"attnT{i}", name=f"attnT{i}") for i in range(2)]
                    rl_rep = [ep.tile([128, 256], F32, tag=f"rlrep{i}", name=f"rlrep{i}") for i in range(2)]
                    for jq in range(NJQ):
                        cs = slice(QG * jq, QG * (jq + 1))
                        for pi in range(4):
                            half, i = pi // 2, pi % 2
                            ti, ro = half, 64 * i
                            nc.sync.dma_start(out=osum[ti][ro:ro + 64, cs],
                                              in_=z_out[jq][pi][0:64, :])
                            nc.sync.dma_start(out=lsum32[ti][ro:ro + 64:32, cs],
                                              in_=z_out[jq][pi][64:66, :])
                            nc.vector.reciprocal(rl32[ti][ro:ro + 64, cs],
                                                 lsum32[ti][ro:ro + 64, cs])
                            for j in range(2):
                                po = ro + 32 * j
                                rb1 = ep.tile([1, QG], F32, name="rbt1", tag="rbt1", bufs=2)
                                rb32 = ep.tile([32, QG], F32, name="rbt32", tag="rbt32", bufs=2)
                                nc.gpsimd.tensor_copy(rb1[:], rl32[ti][po:po + 1, cs])
                                nc.gpsimd.partition_broadcast(rb32[:], rb1[0:1, :])
                                nc.gpsimd.tensor_copy(rl_rep[ti][po:po + 32, cs], rb32[:])
                            nc.vector.tensor_mul(attnT[ti][ro:ro + 64, cs],
                                                 osum[ti][ro:ro + 64, cs],
                                                 rl_rep[ti][ro:ro + 64, cs])

                    # ---- tail epilogue: out-projection, bias, transpose, store ----
                    psum_out = pO.tile([128, 512], F32, tag="O", name="psum_out")
                    for jq in range(NJQ):
                        cs = slice(QG * jq, QG * (jq + 1))
                        for dc in range(2):
                            for m in range(2):
                                nc.tensor.matmul(psum_out[:, 256 * dc + QG * jq:
                                                          256 * dc + QG * (jq + 1)],
                                                 wo_r[m][:, dc * 128:(dc + 1) * 128],
                                                 attnT[m][:, cs], start=(m == 0), stop=(m == 1),
                                                 skip_group_check=True)
                    oT = [ep.tile([128, 256], F32, tag=f"oT{i}", name=f"oT{i}") for i in range(2)]
                    out_sb = [ep.tile([128, 256], F32, tag=f"outsb{i}", name=f"outsb{i}") for i in range(2)]
                    for dc in range(2):
                        nc.vector.tensor_scalar_add(oT[dc][:], psum_out[:, 256 * dc:256 * (dc + 1)],
                                                    bo_c[dc])
                    for qc in range(2):
                        qs = slice(qc * 128, (qc + 1) * 128)
                        pt2 = pO.tile([128, 512], F32, tag="O", name="ptout")
                        for dc in range(2):
                            nc.tensor.transpose(pt2[:, dc * 128:(dc + 1) * 128],
                                                oT[dc][:, qs], ident[:])
                        nc.vector.tensor_copy(out_sb[qc][:], pt2[:, 0:256])
                        nc.sync.dma_start(out=t_out[qc * 128:(qc + 1) * 128, :], in_=out_sb[qc][:])

    nc.compile()
    return nc


_NC_CACHE = {}


def _get_nc():
    if "nc" not in _NC_CACHE:
        _NC_CACHE["nc"] = build_nc()
    return _NC_CACHE["nc"]


def run_cores(inputs, trace=False):
    nc = _get_nc()
    full = {k: np.ascontiguousarray(np.asarray(v, dtype=np.float32)) for k, v in inputs.items()}
    in_maps = []
    for c in range(NC_CORES):
        m = dict(full)
        m["key"] = np.ascontiguousarray(full["key"][c * TLOC:(c + 1) * TLOC])
        m["value"] = np.ascontiguousarray(full["value"][c * TLOC:(c + 1) * TLOC])
        in_maps.append(m)
    res = run_bass_kernel_spmd(nc, in_maps, core_ids=list(range(NC_CORES)), trace=trace)
    out = np.empty((TQ, DOUT), dtype=np.float32)
    for r in range(NC_CORES):
        blk = res.results[r]["out"]
        for jq in range(NJQ):
            q0 = QG * (NC_CORES * jq + r)
            out[q0:q0 + QG, :] = blk[QG * jq:QG * (jq + 1), :]
    return out, res


def kernel(**inputs) -> np.ndarray:
    out, _ = run_cores(inputs, trace=False)
    return out


# revision 12
# speedup vs baseline: 1.4189x; 1.3296x over previous
"""Sequence-parallel attention kernel for 8 Trainium2 NeuronCores.

Problem: nn_Attention_v2 — QKV projections + softmax attention + out-proj.
  query [2048,256], key/value [16384,256], weights [256,256], H=8 heads, KD=VD=32.

Sharding: K/V sequence split 8 ways (2048 rows/core); query replicated.
Each core computes, for all 8 heads, the *unnormalized* attention numerator
Onum = exp(S) @ V and denominator l = exp(S) @ 1 over its local K/V chunk
(no max subtraction needed: logits ~ N(0,1), |S| < ~7, exp is safe in fp32).
A ReduceScatter sums (Onum, l) across cores and shards the result by query
columns; each core then divides, applies the output projection for its query
shard, and the host concatenates the 8 shards.

Fast path vs the original version:
- exp is split between the ACT engine (exact, LUT) and the DVE using a
  3-op bit-trick product approx (two phase-shifted Schraudolph factors,
  ~1.5% max rel err) on 5/16 of key chunks -> ~6e-3 output rel err
  (budget 2e-2): i1 = int32(A*s+B); i2 = i1 + dI; P = bits(i1)*bits(i2).
- All matmul operands are bf16 (weights, transposed inputs, projected
  heads, P): 1-pass PE streams, cheap LDWEIGHTS, and column packing works
  (f32r matmuls require dst partition 0; bf16 doesn't). PSUM accumulation
  stays fp32, so only operand quantization noise (~0.2%) is added.
- AV + softmax-denominator matmuls pack 4-wide per head-pair: numerators
  at psum partitions 0/32 (M=32 each), denominators via ones-column
  matmuls at partitions 64/96 (M=1), all streaming concurrently on
  separate column groups of the PE array.
- Prologue (transpose + project k/q/v) is emitted interleaved with the
  first attention block so the shared PSUM ring never serializes
  attention behind the whole prologue.
"""
import sys

sys.path.insert(0, "/opt/trn_rl_repo")

import numpy as np

import concourse.bass as bass  # noqa: F401  (import order matters)
from concourse import bacc
import concourse.mybir as mybir
from concourse.bass_utils import run_bass_kernel_spmd
from concourse.tile import TileContext
from concourse.masks import make_identity

F32 = mybir.dt.float32
F32R = mybir.dt.float32r
BF16 = mybir.dt.bfloat16
I32 = mybir.dt.int32
EXP = mybir.ActivationFunctionType.Exp
MULT = mybir.AluOpType.mult
ADD = mybir.AluOpType.add

NC_CORES = 8
TQ, T, D = 2048, 16384, 256
H, KD, VD, DOUT = 8, 32, 32, 256
HD = H * KD  # 256
TLOC = T // NC_CORES          # 2048 local K/V rows
NKT = TLOC // 128             # 16 k-chunks
NJQ = TQ // 512               # 4 q-column chunks of 512
QG = 64                       # q columns per rank-group in the RS layout
SCALE = float(1.0 / np.sqrt(KD))

# DVE 3-op exp approximation constants (fitted; ~1.54% max rel err)
A_H = float(np.float32(2**22 * 1.4426950408889634))
B1 = float(np.float32(1066999681.734))
DI = -4193778


def dve_kk_set(b):
    """Which k-chunks of block b the DVE computes exp for (rest on ACT).
    Early blocks are ACT-only: the DVE is busy with prologue transposes."""
    if b < 2:
        return frozenset()
    if b == 2:
        return frozenset({5, 11})
    return frozenset({1, 4, 7, 10, 13})


def build_nc():
    nc = bacc.Bacc("TRN2", target_bir_lowering=False)

    t_query = nc.dram_tensor("query", [TQ, D], F32, kind="ExternalInput")
    t_key = nc.dram_tensor("key", [TLOC, D], F32, kind="ExternalInput")
    t_value = nc.dram_tensor("value", [TLOC, D], F32, kind="ExternalInput")
    t_wq = nc.dram_tensor("wq", [D, HD], F32, kind="ExternalInput")
    t_wk = nc.dram_tensor("wk", [D, HD], F32, kind="ExternalInput")
    t_wv = nc.dram_tensor("wv", [D, HD], F32, kind="ExternalInput")
    t_wo = nc.dram_tensor("wo", [HD, DOUT], F32, kind="ExternalInput")
    t_bq = nc.dram_tensor("bq", [HD], F32, kind="ExternalInput")
    t_bk = nc.dram_tensor("bk", [HD], F32, kind="ExternalInput")
    t_bv = nc.dram_tensor("bv", [HD], F32, kind="ExternalInput")
    t_bo = nc.dram_tensor("bo", [DOUT], F32, kind="ExternalInput")
    t_out = nc.dram_tensor("out", [TQ // NC_CORES, DOUT], F32, kind="ExternalOutput")

    with TileContext(nc) as tc:
        with tc.tile_pool(name="const", bufs=1) as constp, \
             tc.tile_pool(name="persist", bufs=1) as persist, \
             tc.tile_pool(name="dram", bufs=1, space="DRAM") as dramp:

            ident = constp.tile([128, 128], F32)
            make_identity(nc, ident[:])
            ones_f = constp.tile([128, 1], F32)
            nc.gpsimd.memset(ones_f[:], 1.0)
            ones1 = constp.tile([128, 1], BF16)
            nc.vector.tensor_copy(ones1[:], ones_f[:])

            # projected, transposed activations (feature rows on partitions)
            qhT = [persist.tile([128, TQ], BF16, tag=f"qhT{m}", name=f"qhT{m}") for m in range(2)]
            khT = [persist.tile([128, TLOC], BF16, tag=f"khT{m}", name=f"khT{m}") for m in range(2)]
            vh = [persist.tile([128, 256], BF16, tag=f"vh{t}", name=f"vh{t}") for t in range(NKT)]

            with tc.tile_pool(name="tin", bufs=6) as tin, \
                 tc.tile_pool(name="tT", bufs=1) as tTp, \
                 tc.tile_pool(name="pS", bufs=3, space="PSUM") as pS, \
                 tc.tile_pool(name="pO", bufs=2, space="PSUM") as pO, \
                 tc.tile_pool(name="pbuf", bufs=10) as pbuf, \
                 tc.tile_pool(name="ipool", bufs=2) as ipool, \
                 tc.tile_pool(name="wstage", bufs=2) as wstage, \
                 tc.tile_pool(name="stage", bufs=2) as stage:

                def ps_tile():
                    # shared 3-deep PSUM ring: prologue transposes/projections
                    # and the attention S tiles all draw [128,1024] slots
                    return pS.tile([128, 1024], F32, tag="S", name="ps")

                # ---- weights + biases to SBUF (bf16; wq,bq pre-scaled). ----
                # One DMA per weight: [256,256] -> [128, 512] with D-chunk a in
                # cols 256a..; lhsT slice for (dc, m) = [:, 256*dc+128*m :+128].
                wcomb = {}

                def load_w(tdram, key, scale_mul, wdt=F32R):
                    raw = wstage.tile([128, 512], F32, tag="wraw", name="wraw")
                    nc.sync.dma_start(
                        out=raw[:].rearrange("p (a d) -> p a d", a=2),
                        in_=tdram[:].rearrange("(a p) d -> p a d", a=2))
                    wt = persist.tile([128, 512], wdt, tag=f"w_{key}", name=f"w_{key}")
                    if scale_mul is not None:
                        nc.vector.tensor_scalar_mul(wt[:], raw[:], scale_mul)
                    else:
                        nc.vector.tensor_copy(wt[:], raw[:])
                    wcomb[key] = wt
                    return [wt[:, 256 * dc:256 * (dc + 1)] for dc in range(2)]

                def load_b(tdram, dst, scale_mul, key):
                    braw = wstage.tile([128, 2], F32, tag="braw", name="braw")
                    nc.sync.dma_start(out=braw[:],
                                      in_=tdram[:].rearrange("(a p) -> p a", a=2))
                    bt = persist.tile([128, 2], F32, tag=f"b_{key}", name=f"b_{key}")
                    if scale_mul is not None:
                        nc.vector.tensor_scalar_mul(bt[:], braw[:], scale_mul)
                    else:
                        nc.vector.tensor_copy(bt[:], braw[:])
                    for m in range(2):
                        dst[m] = bt[:, m:m + 1]

                # transposed raw inputs in bf16, token-tile-major with the two
                # D-chunks of each 128-token tile adjacent:
                # cols [256*i + 128*dc : ...+128] = tile i, D rows 128dc..
                qT = tTp.tile([128, 2 * TQ], F32R, tag="qT", name="qT")
                kT = tTp.tile([128, 2 * TLOC], F32R, tag="kT", name="kT")
                vT = tTp.tile([128, 2 * TLOC], F32R, tag="vT", name="vT")

                def load_tile(tdram, dst, i, tag):
                    """DMA 128 rows of [tok,256], transpose both 128-col halves
                    into one psum tile, single contiguous copy out (bf16)."""
                    raw = tin.tile([128, 256], F32, tag=f"in_{tag}", name=f"in_{tag}")
                    nc.sync.dma_start(out=raw[:], in_=tdram[i * 128:(i + 1) * 128, :])
                    pt = ps_tile()
                    for m in range(2):
                        nc.tensor.transpose(pt[:, m * 128:(m + 1) * 128],
                                            raw[:, m * 128:(m + 1) * 128], ident[:])
                    nc.vector.tensor_copy(dst[:, 256 * i:256 * (i + 1)], pt[:, 0:256])

                def proj_rhs(src, ntok, j, dc):
                    # tokens 512j..512(j+1), D-chunk dc: 4 strided 128-blocks
                    v = src[:].rearrange("p (i b) -> p i b", b=256)
                    return v[:, 4 * j:4 * (j + 1), 128 * dc:128 * (dc + 1)]

                def kchunk(j):
                    for i in range(4 * j, 4 * j + 4):
                        load_tile(t_key, kT, i, "kT")
                    for m in range(2):
                        pp = ps_tile()
                        for dc in range(2):
                            nc.tensor.matmul(pp[:, 0:512],
                                             wk_r[dc][:, m * 128:(m + 1) * 128],
                                             proj_rhs(kT, TLOC, j, dc),
                                             start=(dc == 0), stop=(dc == 1))
                        nc.vector.tensor_scalar_add(khT[m][:, j * 512:(j + 1) * 512],
                                                    pp[:, 0:512], bk_c[m])

                def qchunk(j):
                    for i in range(4 * j, 4 * j + 4):
                        load_tile(t_query, qT, i, "qT")
                    for m in range(2):
                        pp = ps_tile()
                        for dc in range(2):
                            nc.tensor.matmul(pp[:, 0:512],
                                             wq_r[dc][:, m * 128:(m + 1) * 128],
                                             proj_rhs(qT, TQ, j, dc),
                                             start=(dc == 0), stop=(dc == 1))
                        nc.vector.tensor_scalar_add(qhT[m][:, j * 512:(j + 1) * 512],
                                                    pp[:, 0:512], bq_c[m])

                def vchunk(j):
                    for t in range(4 * j, 4 * j + 4):
                        load_tile(t_value, vT, t, "vT")
                        pp = ps_tile()
                        for dc in range(2):
                            nc.tensor.matmul(pp[:, 0:256],
                                             vT[:, 256 * t + 128 * dc:256 * t + 128 * (dc + 1)],
                                             wv_r[dc][:], start=(dc == 0), stop=(dc == 1))
                        nc.vector.tensor_add(vh[t][:], pp[:, 0:256], bv_rep[:])

                # ---- prologue head: everything block 0 needs to start ----
                wk_r = load_w(t_wk, "wk", None)
                wq_r = load_w(t_wq, "wq", SCALE)
                bq_c, bk_c, bo_c = [None, None], [None, None], [None, None]
                load_b(t_bk, bk_c, None, "bk")
                load_b(t_bq, bq_c, SCALE, "bq")
                wv_r = load_w(t_wv, "wv", None)
                bv_row = persist.tile([1, 256], F32)
                nc.sync.dma_start(out=bv_row[:], in_=t_bv[:].rearrange("(a d) -> a d", a=1))
                bv_rep = persist.tile([128, 256], F32)
                nc.gpsimd.partition_broadcast(bv_rep[:], bv_row[0:1, :])
                kchunk(0)
                qchunk(0)
                vchunk(0)

                # remaining prologue, emitted interleaved into the first blocks
                prologue = []
                for j in range(1, 4):
                    prologue.append(lambda j=j: kchunk(j))
                    prologue.append(lambda j=j: vchunk(j))

                def late_prologue(b):
                    if b == 1:
                        wo_r.extend(load_w(t_wo, "wo", None, BF16))
                        load_b(t_bo, bo_c, None, "bo")
                        qchunk(1)
                    elif b == 2:
                        qchunk(2)
                    elif b == 3:
                        qchunk(3)

                wo_r = []

                # ---- main attention loop + per-(jq, head-pair) ReduceScatter ----
                z_in = [[dramp.tile([NC_CORES, 66, QG], F32, tag=f"zin{j}_{p}", name=f"zin{j}_{p}")
                         for p in range(4)] for j in range(NJQ)]
                z_out = [[dramp.tile([66, QG], F32, tag=f"zout{j}_{p}", name=f"zout{j}_{p}")
                          for p in range(4)] for j in range(NJQ)]

                for jq in range(NJQ):
                    for pi in range(4):
                        b = 4 * jq + pi
                        dve_set = dve_kk_set(b)
                        h0 = 2 * pi
                        ti = h0 // 4
                        po0 = 32 * (h0 % 4)
                        # psO: head h0 rows at partitions 0-31, h1 at 32-63,
                        # softmax denominators at partitions 64 and 96.
                        psO = pO.tile([128, 512], F32, tag="O", name="psO")
                        Pq = []

                        def av_group(kk, P):
                            first, last = kk == 0, kk == NKT - 1
                            for j in range(2):
                                nc.tensor.matmul(
                                    psO[32 * j:32 * j + 32, :],
                                    vh[kk][:, 32 * (h0 + j):32 * (h0 + j) + 32],
                                    P[:, j * 512:(j + 1) * 512],
                                    start=first, stop=last,
                                    tile_position=(0, 32 * j),
                                    skip_group_check=True)
                            for j in range(2):
                                nc.tensor.matmul(
                                    psO[64 + 32 * j:65 + 32 * j, :],
                                    ones1[:],
                                    P[:, j * 512:(j + 1) * 512],
                                    start=first, stop=last,
                                    tile_position=(0, 64 + 32 * j),
                                    skip_group_check=True)

                        for kk in range(NKT):
                            if b == 0 and kk in (4, 8, 12):
                                prologue.pop(0)()   # kchunk(kk//4)
                                prologue.pop(0)()   # vchunk(kk//4)
                            S = ps_tile()
                            for j in range(2):
                                po = po0 + 32 * j
                                nc.tensor.matmul(
                                    S[:, j * 512:(j + 1) * 512],
                                    khT[ti][po:po + 32, kk * 128:(kk + 1) * 128],
                                    qhT[ti][po:po + 32, jq * 512:(jq + 1) * 512],
                                    start=True, stop=True, tile_position=(po, 0))
                            P = pbuf.tile([128, 1024], BF16, tag="P", name="P")
                            if kk in dve_set:
                                i1 = ipool.tile([128, 1024], I32, tag="i1", name="i1")
                                nc.vector.tensor_scalar(i1[:], S[:], A_H, B1, MULT, ADD)
                                i2 = ipool.tile([128, 1024], I32, tag="i2", name="i2")
                                nc.vector.tensor_scalar_add(i2[:], i1[:], DI)
                                nc.vector.tensor_mul(P[:], i1[:].bitcast(F32),
                                                     i2[:].bitcast(F32))
                            else:
                                nc.scalar.activation(P[:], S[:], EXP)
                            Pq.append((kk, P))
                            if len(Pq) > 4:
                                av_group(*Pq.pop(0))
                        while Pq:
                            av_group(*Pq.pop(0))
                        stO = stage.tile([64, 512], F32, tag="stO", name="stO")
                        stL = stage.tile([64, 512], F32, tag="stL", name="stL")
                        nc.scalar.copy(stO[:], psO[0:64, :])
                        nc.scalar.copy(stL[0:1, :], psO[64:65, :])
                        nc.scalar.copy(stL[32:33, :], psO[96:97, :])
                        zi = z_in[jq][pi]
                        nc.sync.dma_start(
                            out=zi[:, 0:64, :].rearrange("r p c -> p r c"),
                            in_=stO[:].rearrange("p (r c) -> p r c", r=NC_CORES))
                        nc.sync.dma_start(
                            out=zi[:, 64:66, :].rearrange("r p c -> p r c"),
                            in_=stL[0:64:32, :].rearrange("p (r c) -> p r c", r=NC_CORES))
                        nc.gpsimd.collective_compute(
                            "ReduceScatter", mybir.AluOpType.add,
                            replica_groups=[list(range(NC_CORES))],
                            ins=[zi.opt()], outs=[z_out[jq][pi].opt()])
                        late_prologue(b)

                # ---- per-(jq,pi) divide as each RS lands (SBUF-only work) ----
                with tc.tile_pool(name="ep", bufs=1) as ep:
                    osum = [ep.tile([128, 256], F32, tag=f"osum{i}", name=f"osum{i}") for i in range(2)]
                    lsum32 = [ep.tile([128, 256], F32, tag=f"lsum32{i}", name=f"lsum32{i}") for i in range(2)]
                    for i in range(2):
                        nc.gpsimd.memset(lsum32[i][:], 1.0)
                    rl32 = [ep.tile([128, 256], F32, tag=f"rl32{i}", name=f"rl32{i}") for i in range(2)]
                    attnT = [ep.tile([128, 256], BF16, tag=f"attnT{i}", name=f"attnT{i}") for i in range(2)]
                    rl_rep = [ep.tile([128, 256], F32, tag=f"rlrep{i}", name=f"rlrep{i}") for i in range(2)]
                    for jq in range(NJQ):
                        cs = slice(QG * jq, QG * (jq + 1))
                        for pi in range(4):
                            half, i = pi // 2, pi % 2
                            ti, ro = half, 64 * i
                            nc.sync.dma_start(out=osum[ti][ro:ro + 64, cs],
                                              in_=z_out[jq][pi][0:64, :])
                            nc.sync.dma_start(out=lsum32[ti][ro:ro + 64:32, cs],
                                              in_=z_out[jq][pi][64:66, :])
                            nc.vector.reciprocal(rl32[ti][ro:ro + 64, cs],
                                                 lsum32[ti][ro:ro + 64, cs])
                            for j in range(2):
                                po = ro + 32 * j
                                rb1 = ep.tile([1, QG], F32, name="rbt1", tag="rbt1", bufs=2)
                                rb32 = ep.tile([32, QG], F32, name="rbt32", tag="rbt32", bufs=2)
                                nc.gpsimd.tensor_copy(rb1[:], rl32[ti][po:po + 1, cs])
                                nc.gpsimd.partition_broadcast(rb32[:], rb1[0:1, :])
                                nc.gpsimd.tensor_copy(rl_rep[ti][po:po + 32, cs], rb32[:])
                            nc.vector.tensor_mul(attnT[ti][ro:ro + 64, cs],
                                                 osum[ti][ro:ro + 64, cs],
                                                 rl_rep[ti][ro:ro + 64, cs])

                    # ---- tail epilogue: out-projection, bias, transpose, store ----
                    psum_out = pO.tile([128, 512], F32, tag="O", name="psum_out")
                    for jq in range(NJQ):
                        cs = slice(QG * jq, QG * (jq + 1))
                        for dc in range(2):
                            for m in range(2):
                                nc.tensor.matmul(psum_out[:, 256 * dc + QG * jq:
                                                          256 * dc + QG * (jq + 1)],
                                                 wo_r[m][:, dc * 128:(dc + 1) * 128],
                                                 attnT[m][:, cs], start=(m == 0), stop=(m == 1),
                                                 skip_group_check=True)
                    oT = [ep.tile([128, 256], F32, tag=f"oT{i}", name=f"oT{i}") for i in range(2)]
                    out_sb = [ep.tile([128, 256], F32, tag=f"outsb{i}", name=f"outsb{i}") for i in range(2)]
                    for dc in range(2):
                        nc.vector.tensor_scalar_add(oT[dc][:], psum_out[:, 256 * dc:256 * (dc + 1)],
                                                    bo_c[dc])
                    for qc in range(2):
                        qs = slice(qc * 128, (qc + 1) * 128)
                        pt2 = pO.tile([128, 512], F32, tag="O", name="ptout")
                        for dc in range(2):
                            nc.tensor.transpose(pt2[:, dc * 128:(dc + 1) * 128],
                                                oT[dc][:, qs], ident[:])
                        nc.vector.tensor_copy(out_sb[qc][:], pt2[:, 0:256])
                        nc.sync.dma_start(out=t_out[qc * 128:(qc + 1) * 128, :], in_=out_sb[qc][:])

    nc.compile()
    return nc


_NC_CACHE = {}


def _get_nc():
    if "nc" not in _NC_CACHE:
        _NC_CACHE["nc"] = build_nc()
    return _NC_CACHE["nc"]


def run_cores(inputs, trace=False):
    nc = _get_nc()
    full = {k: np.ascontiguousarray(np.asarray(v, dtype=np.float32)) for k, v in inputs.items()}
    in_maps = []
    for c in range(NC_CORES):
        m = dict(full)
        m["key"] = np.ascontiguousarray(full["key"][c * TLOC:(c + 1) * TLOC])
        m["value"] = np.ascontiguousarray(full["value"][c * TLOC:(c + 1) * TLOC])
        in_maps.append(m)
    res = run_bass_kernel_spmd(nc, in_maps, core_ids=list(range(NC_CORES)), trace=trace)
    out = np.empty((TQ, DOUT), dtype=np.float32)
    for r in range(NC_CORES):
        blk = res.results[r]["out"]
        for jq in range(NJQ):
            q0 = QG * (NC_CORES * jq + r)
            out[q0:q0 + QG, :] = blk[QG * jq:QG * (jq + 1), :]
    return out, res


def kernel(**inputs) -> np.ndarray:
    out, _ = run_cores(inputs, trace=False)
    return out


# revision 13
# speedup vs baseline: 1.5546x; 1.0956x over previous
"""Sequence-parallel attention kernel for 8 Trainium2 NeuronCores.

Problem: nn_Attention_v2 — QKV projections + softmax attention + out-proj.
  query [2048,256], key/value [16384,256], weights [256,256], H=8 heads, KD=VD=32.

Sharding: K/V sequence split 8 ways (2048 rows/core); query replicated.
Each core computes, for all 8 heads, the *unnormalized* attention numerator
Onum = exp(S) @ V and denominator l = exp(S) @ 1 over its local K/V chunk
(no max subtraction needed: logits ~ N(0,1), |S| < ~7, exp is safe in fp32).
A ReduceScatter sums (Onum, l) across cores and shards the result by query
columns; each core then divides, applies the output projection for its query
shard, and the host concatenates the 8 shards.

Fast path vs the original version:
- exp is split between the ACT engine (exact, LUT) and the DVE using a
  3-op bit-trick product approx (two phase-shifted Schraudolph factors,
  ~1.5% max rel err) on 5/16 of key chunks -> ~6e-3 output rel err
  (budget 2e-2): i1 = int32(A*s+B); i2 = i1 + dI; P = bits(i1)*bits(i2).
- All matmul operands are bf16 (weights, transposed inputs, projected
  heads, P): 1-pass PE streams, cheap LDWEIGHTS, and column packing works
  (f32r matmuls require dst partition 0; bf16 doesn't). PSUM accumulation
  stays fp32, so only operand quantization noise (~0.2%) is added.
- AV + softmax-denominator matmuls pack 4-wide per head-pair: numerators
  at psum partitions 0/32 (M=32 each), denominators via ones-column
  matmuls at partitions 64/96 (M=1), all streaming concurrently on
  separate column groups of the PE array.
- Prologue (transpose + project k/q/v) is emitted interleaved with the
  first attention block so the shared PSUM ring never serializes
  attention behind the whole prologue.
"""
import sys

sys.path.insert(0, "/opt/trn_rl_repo")

import numpy as np

import concourse.bass as bass  # noqa: F401  (import order matters)
from concourse import bacc
import concourse.mybir as mybir
from concourse.bass_utils import run_bass_kernel_spmd
from concourse.tile import TileContext
from concourse.masks import make_identity

F32 = mybir.dt.float32
F32R = mybir.dt.float32r
BF16 = mybir.dt.bfloat16
I32 = mybir.dt.int32
EXP = mybir.ActivationFunctionType.Exp
MULT = mybir.AluOpType.mult
ADD = mybir.AluOpType.add

NC_CORES = 8
TQ, T, D = 2048, 16384, 256
H, KD, VD, DOUT = 8, 32, 32, 256
HD = H * KD  # 256
TLOC = T // NC_CORES          # 2048 local K/V rows
NKT = TLOC // 128             # 16 k-chunks
NJQ = TQ // 512               # 4 q-column chunks of 512
QG = 64                       # q columns per rank-group in the RS layout
SCALE = float(1.0 / np.sqrt(KD))

# DVE 3-op exp approximation constants (fitted; ~1.54% max rel err)
A_H = float(np.float32(2**22 * 1.4426950408889634))
B1 = float(np.float32(1066999681.734))
DI = -4193778


def dve_kk_set(b):
    """Which k-chunks of block b the DVE computes exp for (rest on ACT).
    Early blocks are ACT-only: the DVE is busy with prologue transposes."""
    if b < 2:
        return frozenset()
    if b == 2:
        return frozenset({5, 11})
    return frozenset({1, 4, 7, 10, 13})


def build_nc():
    nc = bacc.Bacc("TRN2", target_bir_lowering=False)

    t_query = nc.dram_tensor("query", [TQ, D], F32, kind="ExternalInput")
    t_key = nc.dram_tensor("key", [TLOC, D], F32, kind="ExternalInput")
    t_value = nc.dram_tensor("value", [TLOC, D], F32, kind="ExternalInput")
    t_wq = nc.dram_tensor("wq", [D, HD], F32, kind="ExternalInput")
    t_wk = nc.dram_tensor("wk", [D, HD], F32, kind="ExternalInput")
    t_wv = nc.dram_tensor("wv", [D, HD], F32, kind="ExternalInput")
    t_wo = nc.dram_tensor("wo", [HD, DOUT], F32, kind="ExternalInput")
    t_bq = nc.dram_tensor("bq", [HD], F32, kind="ExternalInput")
    t_bk = nc.dram_tensor("bk", [HD], F32, kind="ExternalInput")
    t_bv = nc.dram_tensor("bv", [HD], F32, kind="ExternalInput")
    t_bo = nc.dram_tensor("bo", [DOUT], F32, kind="ExternalInput")
    t_out = nc.dram_tensor("out", [TQ // NC_CORES, DOUT], F32, kind="ExternalOutput")

    with TileContext(nc) as tc:
        with tc.tile_pool(name="const", bufs=1) as constp, \
             tc.tile_pool(name="persist", bufs=1) as persist, \
             tc.tile_pool(name="dram", bufs=1, space="DRAM") as dramp:

            ident = constp.tile([128, 128], F32)
            make_identity(nc, ident[:])
            ones_f = constp.tile([128, 1], F32)
            nc.gpsimd.memset(ones_f[:], 1.0)
            ones1 = constp.tile([128, 1], BF16)
            nc.vector.tensor_copy(ones1[:], ones_f[:])

            # projected, transposed activations (feature rows on partitions)
            qhT = [persist.tile([128, TQ], BF16, tag=f"qhT{m}", name=f"qhT{m}") for m in range(2)]
            khT = [persist.tile([128, TLOC], BF16, tag=f"khT{m}", name=f"khT{m}") for m in range(2)]
            vh = [persist.tile([128, 256], BF16, tag=f"vh{t}", name=f"vh{t}") for t in range(NKT)]

            with tc.tile_pool(name="tin", bufs=6) as tin, \
                 tc.tile_pool(name="tT", bufs=1) as tTp, \
                 tc.tile_pool(name="pS", bufs=3, space="PSUM") as pS, \
                 tc.tile_pool(name="pO", bufs=2, space="PSUM") as pO, \
                 tc.tile_pool(name="pbuf", bufs=10) as pbuf, \
                 tc.tile_pool(name="ipool", bufs=2) as ipool, \
                 tc.tile_pool(name="wstage", bufs=2) as wstage, \
                 tc.tile_pool(name="stage", bufs=2) as stage:

                def ps_tile():
                    # shared 3-deep PSUM ring: prologue transposes/projections
                    # and the attention S tiles all draw [128,1024] slots
                    return pS.tile([128, 1024], F32, tag="S", name="ps")

                # ---- weights + biases to SBUF (bf16; wq,bq pre-scaled). ----
                # One DMA per weight: [256,256] -> [128, 512] with D-chunk a in
                # cols 256a..; lhsT slice for (dc, m) = [:, 256*dc+128*m :+128].
                wcomb = {}

                def load_w(tdram, key, scale_mul, wdt=F32R):
                    raw = wstage.tile([128, 512], F32, tag="wraw", name="wraw")
                    nc.sync.dma_start(
                        out=raw[:].rearrange("p (a d) -> p a d", a=2),
                        in_=tdram[:].rearrange("(a p) d -> p a d", a=2))
                    wt = persist.tile([128, 512], wdt, tag=f"w_{key}", name=f"w_{key}")
                    if scale_mul is not None:
                        nc.vector.tensor_scalar_mul(wt[:], raw[:], scale_mul)
                    else:
                        nc.vector.tensor_copy(wt[:], raw[:])
                    wcomb[key] = wt
                    return [wt[:, 256 * dc:256 * (dc + 1)] for dc in range(2)]

                def load_b(tdram, dst, scale_mul, key):
                    braw = wstage.tile([128, 2], F32, tag="braw", name="braw")
                    nc.sync.dma_start(out=braw[:],
                                      in_=tdram[:].rearrange("(a p) -> p a", a=2))
                    bt = persist.tile([128, 2], F32, tag=f"b_{key}", name=f"b_{key}")
                    if scale_mul is not None:
                        nc.vector.tensor_scalar_mul(bt[:], braw[:], scale_mul)
                    else:
                        nc.vector.tensor_copy(bt[:], braw[:])
                    for m in range(2):
                        dst[m] = bt[:, m:m + 1]

                # transposed raw inputs in bf16, token-tile-major with the two
                # D-chunks of each 128-token tile adjacent:
                # cols [256*i + 128*dc : ...+128] = tile i, D rows 128dc..
                qT = tTp.tile([128, 2 * TQ], F32R, tag="qT", name="qT")
                kT = tTp.tile([128, 2 * TLOC], F32R, tag="kT", name="kT")
                vT = tTp.tile([128, 2 * TLOC], F32R, tag="vT", name="vT")

                def load_tile(tdram, dst, i, tag):
                    """DMA 128 rows of [tok,256], transpose both 128-col halves
                    into one psum tile, single contiguous copy out (bf16)."""
                    raw = tin.tile([128, 256], F32, tag=f"in_{tag}", name=f"in_{tag}")
                    nc.sync.dma_start(out=raw[:], in_=tdram[i * 128:(i + 1) * 128, :])
                    pt = ps_tile()
                    for m in range(2):
                        nc.tensor.transpose(pt[:, m * 128:(m + 1) * 128],
                                            raw[:, m * 128:(m + 1) * 128], ident[:])
                    nc.vector.tensor_copy(dst[:, 256 * i:256 * (i + 1)], pt[:, 0:256])

                def proj_rhs(src, ntok, j, dc):
                    # tokens 512j..512(j+1), D-chunk dc: 4 strided 128-blocks
                    v = src[:].rearrange("p (i b) -> p i b", b=256)
                    return v[:, 4 * j:4 * (j + 1), 128 * dc:128 * (dc + 1)]

                def kchunk(j):
                    for i in range(4 * j, 4 * j + 4):
                        load_tile(t_key, kT, i, "kT")
                    for m in range(2):
                        pp = ps_tile()
                        for dc in range(2):
                            nc.tensor.matmul(pp[:, 0:512],
                                             wk_r[dc][:, m * 128:(m + 1) * 128],
                                             proj_rhs(kT, TLOC, j, dc),
                                             start=(dc == 0), stop=(dc == 1))
                        nc.vector.tensor_scalar_add(khT[m][:, j * 512:(j + 1) * 512],
                                                    pp[:, 0:512], bk_c[m])

                def qchunk(j):
                    for i in range(4 * j, 4 * j + 4):
                        load_tile(t_query, qT, i, "qT")
                    for m in range(2):
                        pp = ps_tile()
                        for dc in range(2):
                            nc.tensor.matmul(pp[:, 0:512],
                                             wq_r[dc][:, m * 128:(m + 1) * 128],
                                             proj_rhs(qT, TQ, j, dc),
                                             start=(dc == 0), stop=(dc == 1))
                        nc.vector.tensor_scalar_add(qhT[m][:, j * 512:(j + 1) * 512],
                                                    pp[:, 0:512], bq_c[m])

                def vchunk(j):
                    for t in range(4 * j, 4 * j + 4):
                        load_tile(t_value, vT, t, "vT")
                        pp = ps_tile()
                        for dc in range(2):
                            nc.tensor.matmul(pp[:, 0:256],
                                             vT[:, 256 * t + 128 * dc:256 * t + 128 * (dc + 1)],
                                             wv_r[dc][:], start=(dc == 0), stop=(dc == 1))
                        nc.vector.tensor_add(vh[t][:], pp[:, 0:256], bv_rep[:])

                # ---- prologue head: everything block 0 needs to start ----
                wk_r = load_w(t_wk, "wk", None)
                wq_r = load_w(t_wq, "wq", SCALE)
                bq_c, bk_c, bo_c = [None, None], [None, None], [None, None]
                load_b(t_bk, bk_c, None, "bk")
                load_b(t_bq, bq_c, SCALE, "bq")
                wv_r = load_w(t_wv, "wv", None)
                bv_row = persist.tile([1, 256], F32)
                nc.sync.dma_start(out=bv_row[:], in_=t_bv[:].rearrange("(a d) -> a d", a=1))
                bv_rep = persist.tile([128, 256], F32)
                nc.gpsimd.partition_broadcast(bv_rep[:], bv_row[0:1, :])
                kchunk(0)
                qchunk(0)
                vchunk(0)

                # remaining prologue, emitted interleaved into the first blocks
                prologue = []
                for j in range(1, 4):
                    prologue.append(lambda j=j: kchunk(j))
                    prologue.append(lambda j=j: vchunk(j))

                def late_prologue(b):
                    if b == 1:
                        wo_r.extend(load_w(t_wo, "wo", None, BF16))
                        load_b(t_bo, bo_c, None, "bo")
                        qchunk(1)
                    elif b == 2:
                        qchunk(2)
                    elif b == 3:
                        qchunk(3)

                wo_r = []

                # ---- main attention loop + per-(jq, head-pair) ReduceScatter ----
                z_in = [[dramp.tile([NC_CORES, 66, QG], F32, tag=f"zin{j}_{p}", name=f"zin{j}_{p}")
                         for p in range(4)] for j in range(NJQ)]
                z_out = [[dramp.tile([66, QG], F32, tag=f"zout{j}_{p}", name=f"zout{j}_{p}")
                          for p in range(4)] for j in range(NJQ)]

                Pq = []   # (psO, h0, kk, P) pending AV groups, global stream

                def av_group(psO, h0, kk, P):
                    first, last = kk == 0, kk == NKT - 1
                    for j in range(2):
                        nc.tensor.matmul(
                            psO[32 * j:32 * j + 32, :],
                            vh[kk][:, 32 * (h0 + j):32 * (h0 + j) + 32],
                            P[:, j * 512:(j + 1) * 512],
                            start=first, stop=last,
                            tile_position=(0, 32 * j),
                            skip_group_check=True)
                    for j in range(2):
                        nc.tensor.matmul(
                            psO[64 + 32 * j:65 + 32 * j, :],
                            ones1[:],
                            P[:, j * 512:(j + 1) * 512],
                            start=first, stop=last,
                            tile_position=(0, 64 + 32 * j),
                            skip_group_check=True)

                def stage_rs(psO, jq, pi):
                    stO = stage.tile([64, 512], F32, tag="stO", name="stO")
                    stL = stage.tile([64, 512], F32, tag="stL", name="stL")
                    nc.scalar.copy(stO[:], psO[0:64, :])
                    nc.scalar.copy(stL[0:1, :], psO[64:65, :])
                    nc.scalar.copy(stL[32:33, :], psO[96:97, :])
                    zi = z_in[jq][pi]
                    nc.sync.dma_start(
                        out=zi[:, 0:64, :].rearrange("r p c -> p r c"),
                        in_=stO[:].rearrange("p (r c) -> p r c", r=NC_CORES))
                    nc.sync.dma_start(
                        out=zi[:, 64:66, :].rearrange("r p c -> p r c"),
                        in_=stL[0:64:32, :].rearrange("p (r c) -> p r c", r=NC_CORES))
                    nc.gpsimd.collective_compute(
                        "ReduceScatter", mybir.AluOpType.add,
                        replica_groups=[list(range(NC_CORES))],
                        ins=[zi.opt()], outs=[z_out[jq][pi].opt()])

                done_blocks = []   # (psO, jq, pi) whose last AV has been emitted

                for jq in range(NJQ):
                    for pi in range(4):
                        b = 4 * jq + pi
                        dve_set = dve_kk_set(b)
                        h0 = 2 * pi
                        ti = h0 // 4
                        po0 = 32 * (h0 % 4)
                        # psO: head h0 rows at partitions 0-31, h1 at 32-63,
                        # softmax denominators at partitions 64 and 96.
                        psO = pO.tile([128, 512], F32, tag="O", name="psO")
                        for kk in range(NKT):
                            if b == 0 and kk in (4, 8, 12):
                                prologue.pop(0)()   # kchunk(kk//4)
                                prologue.pop(0)()   # vchunk(kk//4)
                            S = ps_tile()
                            for j in range(2):
                                po = po0 + 32 * j
                                nc.tensor.matmul(
                                    S[:, j * 512:(j + 1) * 512],
                                    khT[ti][po:po + 32, kk * 128:(kk + 1) * 128],
                                    qhT[ti][po:po + 32, jq * 512:(jq + 1) * 512],
                                    start=True, stop=True, tile_position=(po, 0))
                            P = pbuf.tile([128, 1024], BF16, tag="P", name="P")
                            if kk in dve_set:
                                i1 = ipool.tile([128, 1024], I32, tag="i1", name="i1")
                                nc.vector.tensor_scalar(i1[:], S[:], A_H, B1, MULT, ADD)
                                i2 = ipool.tile([128, 1024], I32, tag="i2", name="i2")
                                nc.vector.tensor_scalar_add(i2[:], i1[:], DI)
                                nc.vector.tensor_mul(P[:], i1[:].bitcast(F32),
                                                     i2[:].bitcast(F32))
                            else:
                                nc.scalar.activation(P[:], S[:], EXP)
                            Pq.append((psO, h0, kk, P))
                            if len(Pq) > 4:
                                av_group(*Pq.pop(0))
                                if Pq[0][2] == 0 and done_blocks:
                                    # previous block's last AV just went out
                                    stage_rs(*done_blocks.pop(0))
                        done_blocks.append((psO, jq, pi))
                        late_prologue(b)
                while Pq:
                    av_group(*Pq.pop(0))
                while done_blocks:
                    stage_rs(*done_blocks.pop(0))

                # ---- per-(jq,pi) divide as each RS lands (SBUF-only work) ----
                with tc.tile_pool(name="ep", bufs=1) as ep:
                    osum = [ep.tile([128, 256], F32, tag=f"osum{i}", name=f"osum{i}") for i in range(2)]
                    lsum32 = [ep.tile([128, 256], F32, tag=f"lsum32{i}", name=f"lsum32{i}") for i in range(2)]
                    for i in range(2):
                        nc.gpsimd.memset(lsum32[i][:], 1.0)
                    rl32 = [ep.tile([128, 256], F32, tag=f"rl32{i}", name=f"rl32{i}") for i in range(2)]
                    attnT = [ep.tile([128, 256], BF16, tag=f"attnT{i}", name=f"attnT{i}") for i in range(2)]
                    rl_rep = [ep.tile([128, 256], F32, tag=f"rlrep{i}", name=f"rlrep{i}") for i in range(2)]
                    for jq in range(NJQ):
                        cs = slice(QG * jq, QG * (jq + 1))
                        for pi in range(4):
                            half, i = pi // 2, pi % 2
                            ti, ro = half, 64 * i
                            nc.sync.dma_start(out=osum[ti][ro:ro + 64, cs],
                                              in_=z_out[jq][pi][0:64, :])
                            nc.sync.dma_start(out=lsum32[ti][ro:ro + 64:32, cs],
                                              in_=z_out[jq][pi][64:66, :])
                            nc.vector.reciprocal(rl32[ti][ro:ro + 64, cs],
                                                 lsum32[ti][ro:ro + 64, cs])
                            for j in range(2):
                                po = ro + 32 * j
                                rb1 = ep.tile([1, QG], F32, name="rbt1", tag="rbt1", bufs=2)
                                rb32 = ep.tile([32, QG], F32, name="rbt32", tag="rbt32", bufs=2)
                                nc.gpsimd.tensor_copy(rb1[:], rl32[ti][po:po + 1, cs])
                                nc.gpsimd.partition_broadcast(rb32[:], rb1[0:1, :])
                                nc.gpsimd.tensor_copy(rl_rep[ti][po:po + 32, cs], rb32[:])
                            nc.vector.tensor_mul(attnT[ti][ro:ro + 64, cs],
                                                 osum[ti][ro:ro + 64, cs],
                                                 rl_rep[ti][ro:ro + 64, cs])

                    # ---- tail epilogue: out-projection, bias, transpose, store ----
                    psum_out = pO.tile([128, 512], F32, tag="O", name="psum_out")
                    for jq in range(NJQ):
                        cs = slice(QG * jq, QG * (jq + 1))
                        for dc in range(2):
                            for m in range(2):
                                nc.tensor.matmul(psum_out[:, 256 * dc + QG * jq:
                                                          256 * dc + QG * (jq + 1)],
                                                 wo_r[m][:, dc * 128:(dc + 1) * 128],
                                                 attnT[m][:, cs], start=(m == 0), stop=(m == 1),
                                                 skip_group_check=True)
                    oT = [ep.tile([128, 256], F32, tag=f"oT{i}", name=f"oT{i}") for i in range(2)]
                    out_sb = [ep.tile([128, 256], F32, tag=f"outsb{i}", name=f"outsb{i}") for i in range(2)]
                    for dc in range(2):
                        nc.vector.tensor_scalar_add(oT[dc][:], psum_out[:, 256 * dc:256 * (dc + 1)],
                                                    bo_c[dc])
                    for qc in range(2):
                        qs = slice(qc * 128, (qc + 1) * 128)
                        pt2 = pO.tile([128, 512], F32, tag="O", name="ptout")
                        for dc in range(2):
                            nc.tensor.transpose(pt2[:, dc * 128:(dc + 1) * 128],
                                                oT[dc][:, qs], ident[:])
                        nc.vector.tensor_copy(out_sb[qc][:], pt2[:, 0:256])
                        nc.sync.dma_start(out=t_out[qc * 128:(qc + 1) * 128, :], in_=out_sb[qc][:])

    nc.compile()
    return nc


_NC_CACHE = {}


def _get_nc():
    if "nc" not in _NC_CACHE:
        _NC_CACHE["nc"] = build_nc()
    return _NC_CACHE["nc"]


def run_cores(inputs, trace=False):
    nc = _get_nc()
    full = {k: np.ascontiguousarray(np.asarray(v, dtype=np.float32)) for k, v in inputs.items()}
    in_maps = []
    for c in range(NC_CORES):
        m = dict(full)
        m["key"] = np.ascontiguousarray(full["key"][c * TLOC:(c + 1) * TLOC])
        m["value"] = np.ascontiguousarray(full["value"][c * TLOC:(c + 1) * TLOC])
        in_maps.append(m)
    res = run_bass_kernel_spmd(nc, in_maps, core_ids=list(range(NC_CORES)), trace=trace)
    out = np.empty((TQ, DOUT), dtype=np.float32)
    for r in range(NC_CORES):
        blk = res.results[r]["out"]
        for jq in range(NJQ):
            q0 = QG * (NC_CORES * jq + r)
            out[q0:q0 + QG, :] = blk[QG * jq:QG * (jq + 1), :]
    return out, res


def kernel(**inputs) -> np.ndarray:
    out, _ = run_cores(inputs, trace=False)
    return out
